# revision 1
# baseline (speedup 1.0000x reference)
"""CosmosTransformerBlock on 8 Trainium2 NeuronCores.

Strategy (tensor-parallel + sequence-parallel residual):
- Attention heads (16) sharded 2/core; FF inner dim (8192) sharded 1024/core.
- Column-parallel wq/wk/wv/ff_w1, row-parallel wo/ff_w2 -> partial outputs
  ReduceScatter'd over the sequence dim (each core owns 256 tokens of the
  residual stream h).
- LayerNorm / AdaLN modulation computed on the local 256-token slice, then
  the modulated activations are transposed to feature-major layout and
  AllGather'd so every core has the full [hid, seq] activation for its
  matmul shards.
- All matmuls run in float32r (fp32 rounded to 12-bit mantissa; full PE
  throughput at moving-dim >= 256), accumulating in fp32 PSUM.
"""

import sys

import numpy as np

try:
    import concourse.bass as bass
except ImportError:  # pragma: no cover
    sys.path.insert(0, "/opt/trn_rl_repo")
    import concourse.bass as bass

import concourse.mybir as mybir
import concourse.tile as tile
from concourse import bacc
from concourse.bass_utils import run_bass_kernel_spmd

F32 = mybir.dt.float32
F32R = mybir.dt.float32r
AF = mybir.ActivationFunctionType
ALU = mybir.AluOpType

NCORES = 8
S = 2048          # sequence length
C = 2048          # hidden dim
SL = S // NCORES  # 256 tokens per core
D = 128           # head dim
HL = 2            # local heads per core
CROSS = 1024
SC = 512          # encoder sequence length
LORA = 256
FFL = 8192 // NCORES  # 1024 ff dims per core
CT = C // 128     # 16 c tiles
EPS = 1e-6
ISQD = float(D) ** -0.5


def round_fp32r(a: np.ndarray) -> np.ndarray:
    """Round fp32 to fp32r (12-bit mantissa, RNE) — what the PE consumes."""
    b = np.ascontiguousarray(a, dtype=np.float32).view(np.uint32)
    lsb = (b >> np.uint32(12)) & np.uint32(1)
    r = (b + np.uint32(0x7FF) + lsb) & np.uint32(0xFFFFF000)
    return r.view(np.float32)


def _build(nc: bacc.Bacc):
    dram = lambda n, s, d: nc.dram_tensor(n, s, d, kind="ExternalInput").ap()

    h_in = dram("h_s", [SL, C], F32R)
    wq1 = dram("wq1", [C, HL * D], F32R)
    wk1 = dram("wk1", [C, HL * D], F32R)
    wv1 = dram("wv1", [C, HL * D], F32R)
    wo1 = dram("wo1", [HL * D, C], F32R)
    wq2 = dram("wq2", [C, HL * D], F32R)
    wk2 = dram("wk2", [CROSS, HL * D], F32R)
    wv2 = dram("wv2", [CROSS, HL * D], F32R)
    wo2 = dram("wo2", [HL * D, C], F32R)
    encT = dram("encT", [CROSS, SC], F32R)
    w1f = dram("w1f", [C, FFL], F32R)
    w2f = dram("w2f", [FFL, C], F32R)
    aw1 = dram("aw1", [3, C, LORA], F32R)
    aw2 = dram("aw2", [3, LORA, 3 * C], F32R)
    emb_t = dram("emb_t", [128, CT], F32)      # embedded_timestep, scattered
    temb_r = dram("temb_r", [1, 3 * C], F32)
    cosT = dram("cosT", [D, S], F32)
    sinT = dram("sinT", [D, S], F32)
    rotm = dram("rotm", [D, D], F32R)          # lhsT of the rotate-half perm
    eye = dram("eye", [128, 128], F32R)
    onec = dram("onec", [128, 1], F32R)

    h_out = nc.dram_tensor("h_out", [SL, C], F32R, kind="ExternalOutput").ap()

    # internal DRAM (collectives chunked 4x over the feature dim)
    rs_in = [[nc.dram_tensor(f"rs_in{L}_{cc}", [S, 512], F32).ap()
              for cc in range(4)] for L in range(3)]
    rs_out = [[nc.dram_tensor(f"rs_out{L}_{cc}", [SL, 512], F32).ap()
               for cc in range(4)] for L in range(3)]
    ag_in = [nc.dram_tensor(f"ag_in{L}", [C, SL], F32R).ap() for L in range(3)]
    ag_out = [nc.dram_tensor(f"ag_out{L}", [4 * NCORES * 512, SL], F32R,
                             addr_space="Shared").ap() for L in range(3)]

    RG = [list(range(NCORES))]

    with tile.TileContext(nc) as tc:
        ctx_pool = tc.tile_pool(name="persist", bufs=1)
        persist = ctx_pool.__enter__()

        h_sb = persist.tile([128, 2, C], F32R, tag="h")
        nc.sync.dma_start(out=h_sb, in_=h_in.rearrange("(u p) c -> p u c", p=128))

        onec_sb = persist.tile([128, 1], F32R, tag="onec")
        nc.sync.dma_start(out=onec_sb, in_=onec)
        eye_sb = persist.tile([128, 128], F32R, tag="eye")
        nc.sync.dma_start(out=eye_sb, in_=eye)
        rotm_sb = persist.tile([D, D], F32R, tag="rotm")
        nc.sync.dma_start(out=rotm_sb, in_=rotm)
        emb_sb = persist.tile([128, CT], F32, tag="emb")
        nc.sync.dma_start(out=emb_sb, in_=emb_t)
        eps_sb = persist.tile([128, 1], F32, tag="eps")
        nc.vector.memset(eps_sb, EPS)

        # modulation vectors for the current sublayer, broadcast to 128 parts
        shB = persist.tile([128, C], F32, tag="shB")
        opscB = persist.tile([128, C], F32, tag="opscB")
        gateB = persist.tile([128, C], F32, tag="gateB")

        # ---------------- AdaLN: e_L = silu(emb) @ w1_L @ w2_L + temb -------
        silu_sb = persist.tile([128, CT], F32R, tag="silu")
        nc.scalar.activation(out=silu_sb, in_=emb_sb, func=AF.Silu)

        e_dram = nc.dram_tensor("e_dram", [3, 3 * C], F32).ap()

        with tc.tile_pool(name="adaln", bufs=1) as apool, \
             tc.tile_pool(name="adaln_ps", bufs=2, space="PSUM") as apsum, \
             tc.tile_pool(name="adaln_dram", bufs=1, space="DRAM") as adram:
            temb_sb = apool.tile([1, 3 * C], F32, tag="temb")
            nc.sync.dma_start(out=temb_sb, in_=temb_r)
            for L in range(3):
                aw1_sb = apool.tile([128, CT, LORA], F32R, tag="aw1")
                nc.sync.dma_start(
                    out=aw1_sb,
                    in_=aw1[L].rearrange("(t p) m -> p t m", p=128))
                y_ps = apsum.tile([1, LORA], F32, tag="yps")
                for ct in range(CT):
                    nc.tensor.matmul(y_ps, lhsT=silu_sb[:, ct:ct + 1],
                                     rhs=aw1_sb[:, ct, :],
                                     start=(ct == 0), stop=(ct == CT - 1))
                y_sb = apool.tile([1, LORA], F32R, tag="y")
                nc.scalar.copy(out=y_sb, in_=y_ps)
                yb = adram.tile([1, LORA], F32R, tag="yb")
                nc.sync.dma_start(out=yb, in_=y_sb)
                y2 = apool.tile([128, 2], F32R, tag="y2")
                nc.sync.dma_start(
                    out=y2, in_=yb.rearrange("o (t p) -> p (o t)", p=128))

                aw2_sb = apool.tile([128, 2, 12, 512], F32R, tag="aw2")
                nc.sync.dma_start(
                    out=aw2_sb,
                    in_=aw2[L].rearrange("(t p) (ch n) -> p t ch n",
                                         p=128, n=512))
                for ch in range(12):
                    e_ps = apsum.tile([1, 512], F32, tag="e_ps")
                    nc.tensor.matmul(e_ps, lhsT=y2[:, 0:1],
                                     rhs=aw2_sb[:, 0, ch, :],
                                     start=True, stop=False)
                    nc.tensor.matmul(e_ps, lhsT=y2[:, 1:2],
                                     rhs=aw2_sb[:, 1, ch, :],
                                     start=False, stop=True)
                    e_row = apool.tile([1, 512], F32, tag="e_row")
                    nc.vector.tensor_add(
                        out=e_row,
                        in0=e_ps,
                        in1=temb_sb[:, 512 * ch:512 * (ch + 1)])
                    nc.sync.dma_start(
                        out=e_dram[L:L + 1, 512 * ch:512 * (ch + 1)],
                        in_=e_row)

        def _ebcast(L, off):
            # DRAM partition-step-0 broadcast AP of e_dram[L, off:off+C]
            return bass.AP(tensor=e_dram.tensor, offset=L * 3 * C + off,
                           ap=[[0, 128], [1, C]])

        def load_modvecs(L):
            nc.sync.dma_start(out=shB, in_=_ebcast(L, 0))
            nc.sync.dma_start(out=opscB, in_=_ebcast(L, C))
            nc.vector.tensor_scalar_add(out=opscB, in0=opscB, scalar1=1.0)
            nc.sync.dma_start(out=gateB, in_=_ebcast(L, 2 * C))

        # ---------------- LN + modulate + transpose + AllGather -------------
        def modulate(L, mpool, mpsum):
            load_modvecs(L)
            xns = []
            for u in range(2):
                x = h_sb[:, u, :]
                stats = mpool.tile([128, 4, 6], F32, tag="stats")
                xv = x.rearrange("p (g n) -> p g n", n=512)
                for g in range(4):
                    nc.vector.bn_stats(out=stats[:, g, :], in_=xv[:, g, :])
                mv = mpool.tile([128, 2], F32, tag="mv")
                nc.vector.bn_aggr(out=mv, in_=stats)
                sd = mpool.tile([128, 1], F32, tag="sd")
                nc.scalar.activation(out=sd, in_=mv[:, 1:2], func=AF.Sqrt,
                                     bias=eps_sb)
                rstd = mpool.tile([128, 1], F32, tag="rstd")
                nc.vector.reciprocal(out=rstd, in_=sd)
                t1 = mpool.tile([128, C], F32, tag="mod_t1")
                nc.vector.tensor_scalar(out=t1, in0=x, scalar1=mv[:, 0:1],
                                        scalar2=rstd, op0=ALU.subtract,
                                        op1=ALU.mult)
                t2 = mpool.tile([128, C], F32, tag="mod_t2")
                nc.vector.tensor_mul(out=t2, in0=t1, in1=opscB)
                xn = mpool.tile([128, C], F32R, tag="mod_xn", name=f"xn{u}")
                nc.vector.tensor_add(out=xn, in0=t2, in1=shB)
                xns.append(xn)
            agw = ag_in[L].rearrange("(t p) (u q) -> p t u q", p=128, q=128)
            for cc in range(4):
                for tl in range(4):
                    ct = 4 * cc + tl
                    for u in range(2):
                        tp = mpsum.tile([128, 128], F32R, tag="tpsum")
                        nc.tensor.transpose(
                            tp, xns[u][:, 128 * ct:128 * (ct + 1)], eye_sb)
                        xe = mpool.tile([128, 128], F32R, tag="mod_xe")
                        nc.vector.tensor_copy(out=xe, in_=tp)
                        nc.sync.dma_start(out=agw[:, ct, u, :], in_=xe)
                nc.gpsimd.collective_compute(
                    "AllGather", ALU.bypass, replica_groups=RG,
                    ins=[ag_in[L][512 * cc:512 * (cc + 1), :]],
                    outs=[ag_out[L][4096 * cc:4096 * (cc + 1), :]])

        # xn reader view: [p, cchunk, tlocal, rank, s_loc]
        def agv_of(L):
            return ag_out[L].rearrange("(cc r t p) s -> p cc t r s",
                                       p=128, r=NCORES, cc=4)

        # rms(+rope) epilogue: ps [128, 512] -> dst [128, 512] f32r
        def rms_epilogue(ps, dst, ch, rope, pool, psum):
            sq = pool.tile([128, 512], F32R, tag="ep_sq")
            nc.scalar.activation(out=sq, in_=ps, func=AF.Square)
            ss = psum.tile([1, 512], F32, tag="ep_ss")
            nc.tensor.matmul(ss, lhsT=onec_sb, rhs=sq, start=True, stop=True)
            sd = pool.tile([1, 512], F32, tag="ep_sd")
            nc.scalar.activation(out=sd, in_=ss, func=AF.Sqrt,
                                 scale=1.0 / D, bias=eps_sb[0:1, :])
            rstd = pool.tile([1, 512], F32, tag="ep_rstd")
            nc.vector.reciprocal(out=rstd, in_=sd)
            rb = pool.tile([128, 512], F32, tag="ep_rb")
            nc.gpsimd.partition_broadcast(out_ap=rb, in_ap=rstd)
            if rope:
                qe = pool.tile([128, 512], F32R, tag="ep_qe")
                nc.scalar.copy(out=qe, in_=ps)
                rot = psum.tile([128, 512], F32, tag="ep_rot")
                nc.tensor.matmul(rot, lhsT=rotm_sb, rhs=qe,
                                 start=True, stop=True)
                t1 = pool.tile([128, 512], F32, tag="ep_t1")
                nc.vector.tensor_mul(out=t1, in0=qe,
                                     in1=cos_sb[:, 512 * ch:512 * (ch + 1)])
                t2 = pool.tile([128, 512], F32, tag="ep_t2")
                nc.vector.tensor_mul(out=t2, in0=rot,
                                     in1=sin_sb[:, 512 * ch:512 * (ch + 1)])
                nc.vector.tensor_add(out=t1, in0=t1, in1=t2)
                nc.vector.tensor_mul(out=dst, in0=t1, in1=rb)
            else:
                nc.vector.tensor_mul(out=dst, in0=ps, in1=rb)

        # sdpa for one (chunk, head): writes normalized A^T [d, 512] f32r
        def sdpa(q_sb, k_sb, v_sb, nkt, h, ch, dst, pool, psum):
            av = psum.tile([128, 512], F32, tag="sd_av")
            den = psum.tile([1, 512], F32, tag="sd_den")
            for kt in range(nkt):
                sps = psum.tile([128, 512], F32, tag="sd_s")
                nc.tensor.matmul(
                    sps, lhsT=k_sb[:, h, 128 * kt:128 * (kt + 1)],
                    rhs=q_sb[:, h, 512 * ch:512 * (ch + 1)],
                    start=True, stop=True)
                pt = pool.tile([128, 512], F32R, tag="sd_pt")
                nc.scalar.activation(out=pt, in_=sps, func=AF.Exp, scale=ISQD)
                nc.tensor.matmul(den, lhsT=onec_sb, rhs=pt,
                                 start=(kt == 0), stop=(kt == nkt - 1))
                nc.tensor.matmul(av, lhsT=v_sb[:, kt, 128 * h:128 * (h + 1)],
                                 rhs=pt, start=(kt == 0), stop=(kt == nkt - 1))
            rec = pool.tile([1, 512], F32, tag="sd_rec")
            nc.vector.reciprocal(out=rec, in_=den)
            rb = pool.tile([128, 512], F32, tag="sd_rb")
            nc.gpsimd.partition_broadcast(out_ap=rb, in_ap=rec)
            nc.vector.tensor_mul(out=dst, in0=av, in1=rb)

        # O = A @ wo (one chunk of q): psum [s_tile, c_chunk], evict, DMA
        def wo_phase(a0, a1, wo_sb, ch, L, pool, psum):
            for ss in range(4):
                st = 4 * ch + ss
                for cc in range(4):
                    ops = psum.tile([128, 512], F32, tag="wo_ps")
                    nc.tensor.matmul(
                        ops, lhsT=a0[:, 128 * ss:128 * (ss + 1)],
                        rhs=wo_sb[:, 0, 512 * cc:512 * (cc + 1)],
                        start=True, stop=False)
                    nc.tensor.matmul(
                        ops, lhsT=a1[:, 128 * ss:128 * (ss + 1)],
                        rhs=wo_sb[:, 1, 512 * cc:512 * (cc + 1)],
                        start=False, stop=True)
                    oe = pool.tile([128, 512], F32, tag="wo_oe")
                    nc.vector.tensor_copy(out=oe, in_=ops)
                    rsv = rs_in[L][cc].rearrange("(st p) c -> p st c", p=128)
                    nc.sync.dma_start(out=rsv[:, st, :], in_=oe)

        def reduce_and_residual(L):
            for cc in range(4):
                nc.gpsimd.collective_compute(
                    "ReduceScatter", ALU.add, replica_groups=RG,
                    ins=[rs_in[L][cc]], outs=[rs_out[L][cc]])
            with tc.tile_pool(name=f"res{L}", bufs=2) as rpool:
                for cc in range(4):
                    rov = rs_out[L][cc].rearrange("(u p) c -> p u c", p=128)
                    for u in range(2):
                        ot = rpool.tile([128, 512], F32, tag="res_o")
                        nc.sync.dma_start(out=ot, in_=rov[:, u, :])
                        gt = rpool.tile([128, 512], F32, tag="res_g")
                        nc.vector.tensor_mul(
                            out=gt, in0=ot,
                            in1=gateB[:, 512 * cc:512 * (cc + 1)])
                        nc.vector.tensor_add(
                            out=h_sb[:, u, 512 * cc:512 * (cc + 1)],
                            in0=h_sb[:, u, 512 * cc:512 * (cc + 1)], in1=gt)

        # ======================= sublayer 0: self attention ================
        with tc.tile_pool(name="mod0", bufs=2) as mpool, \
             tc.tile_pool(name="mod0ps", bufs=2, space="PSUM") as mpsum:
            modulate(0, mpool, mpsum)

        with tc.tile_pool(name="attn1", bufs=1) as apool:
            q_sb = apool.tile([D, HL, S], F32R, tag="q")
            k_sb = apool.tile([D, HL, S], F32R, tag="k")
            v_sb = apool.tile([128, CT, HL * D], F32R, tag="v")
            agv = agv_of(0)

            with tc.tile_pool(name="qkv1", bufs=1) as wpool, \
                 tc.tile_pool(name="qkv1x", bufs=4) as xpool, \
                 tc.tile_pool(name="qkv1w", bufs=1) as epool, \
                 tc.tile_pool(name="qkv1ps", bufs=1, space="PSUM") as qpsum, \
                 tc.tile_pool(name="qkv1ps2", bufs=1, space="PSUM") as qpsum2:
                cos_sb = apool.tile([D, S], F32, tag="cos")
                nc.sync.dma_start(out=cos_sb, in_=cosT)
                sin_sb = apool.tile([D, S], F32, tag="sin")
                nc.sync.dma_start(out=sin_sb, in_=sinT)
                wq_sb = wpool.tile([128, CT, HL * D], F32R, tag="wq")
                nc.sync.dma_start(out=wq_sb,
                                  in_=wq1.rearrange("(t p) m -> p t m", p=128))
                wk_sb = wpool.tile([128, CT, HL * D], F32R, tag="wk")
                nc.sync.dma_start(out=wk_sb,
                                  in_=wk1.rearrange("(t p) m -> p t m", p=128))
                wv_sb = wpool.tile([128, CT, HL * D], F32R, tag="wv")
                nc.sync.dma_start(out=wv_sb,
                                  in_=wv1.rearrange("(t p) m -> p t m", p=128))
                vT_sb = wpool.tile([D, HL, S], F32R, tag="vT")
                for ch in range(4):
                    pq = [qpsum.tile([128, 512], F32, tag=f"pq{j}", name=f"pq{j}")
                          for j in range(2)]
                    pk = [qpsum.tile([128, 512], F32, tag=f"pk{j}", name=f"pk{j}")
                          for j in range(2)]
                    pv = [qpsum.tile([128, 512], F32, tag=f"pv{j}", name=f"pv{j}")
                          for j in range(2)]
                    for ct in range(CT):
                        st, sp = (ct == 0), (ct == CT - 1)
                        xt = xpool.tile([128, 512], F32R, tag="xt")
                        nc.sync.dma_start(out=xt,
                                          in_=agv[:, ct // 4, ct % 4, 2 * ch:2 * ch + 2, :])
                        for hh in range(HL):
                            nc.tensor.matmul(
                                pq[hh],
                                lhsT=wq_sb[:, ct, 128 * hh:128 * (hh + 1)],
                                rhs=xt, start=st, stop=sp)
                            nc.tensor.matmul(
                                pk[hh],
                                lhsT=wk_sb[:, ct, 128 * hh:128 * (hh + 1)],
                                rhs=xt, start=st, stop=sp)
                            nc.tensor.matmul(
                                pv[hh],
                                lhsT=wv_sb[:, ct, 128 * hh:128 * (hh + 1)],
                                rhs=xt, start=st, stop=sp)
                    for hh in range(HL):
                        nc.vector.tensor_copy(
                            out=vT_sb[:, hh, 512 * ch:512 * (ch + 1)],
                            in_=pv[hh])
                        rms_epilogue(pq[hh], q_sb[:, hh, 512 * ch:512 * (ch + 1)],
                                     ch, True, epool, qpsum2)
                        rms_epilogue(pk[hh], k_sb[:, hh, 512 * ch:512 * (ch + 1)],
                                     ch, True, epool, qpsum2)
                # V^T -> V (k-major) for the AV matmuls
                for hh in range(HL):
                    for kt in range(CT):
                        vtp = qpsum2.tile([128, 128], F32R, tag="ep_rot",
                                          name="vtp")
                        nc.tensor.transpose(
                            vtp, vT_sb[:, hh, 128 * kt:128 * (kt + 1)], eye_sb)
                        nc.vector.tensor_copy(
                            out=v_sb[:, kt, 128 * hh:128 * (hh + 1)], in_=vtp)

            with tc.tile_pool(name="sdpa1", bufs=2) as spool, \
                 tc.tile_pool(name="sdpa1ps", bufs=2, space="PSUM") as spsum:
                wo_sb = spool.tile([128, 2, C], F32R, tag="wo", bufs=1)
                nc.sync.dma_start(out=wo_sb,
                                  in_=wo1.rearrange("(t p) m -> p t m", p=128))
                for ch in range(4):
                    a0 = spool.tile([D, 512], F32R, tag="a0")
                    a1 = spool.tile([D, 512], F32R, tag="a1")
                    sdpa(q_sb, k_sb, v_sb, CT, 0, ch, a0, spool, spsum)
                    sdpa(q_sb, k_sb, v_sb, CT, 1, ch, a1, spool, spsum)
                    wo_phase(a0, a1, wo_sb, ch, 0, spool, spsum)

        reduce_and_residual(0)

        # ======================= sublayer 1: cross attention ===============
        with tc.tile_pool(name="mod1", bufs=2) as mpool, \
             tc.tile_pool(name="mod1ps", bufs=2, space="PSUM") as mpsum:
            modulate(1, mpool, mpsum)

        with tc.tile_pool(name="attn2", bufs=1) as apool:
            q2_sb = apool.tile([D, HL, S], F32R, tag="q2")
            k2_sb = apool.tile([D, HL, SC], F32R, tag="k2")
            v2_sb = apool.tile([128, 4, HL * D], F32R, tag="v2")
            agv = agv_of(1)

            with tc.tile_pool(name="kv2", bufs=1) as wpool, \
                 tc.tile_pool(name="kv2e", bufs=1) as epool, \
                 tc.tile_pool(name="kv2ps", bufs=1, space="PSUM") as qpsum, \
                 tc.tile_pool(name="kv2ps2", bufs=1, space="PSUM") as qpsum2:
                enc_sb = wpool.tile([128, 8, SC], F32R, tag="enc")
                nc.sync.dma_start(out=enc_sb,
                                  in_=encT.rearrange("(t p) s -> p t s", p=128))
                wk2_sb = wpool.tile([128, 8, HL * D], F32R, tag="wk2")
                nc.sync.dma_start(out=wk2_sb,
                                  in_=wk2.rearrange("(t p) m -> p t m", p=128))
                wv2_sb = wpool.tile([128, 8, HL * D], F32R, tag="wv2")
                nc.sync.dma_start(out=wv2_sb,
                                  in_=wv2.rearrange("(t p) m -> p t m", p=128))
                vT2_sb = wpool.tile([D, HL, SC], F32R, tag="vT2")
                for hh in range(HL):
                    pk2 = qpsum.tile([128, SC], F32, tag="pk2")
                    pv2 = qpsum.tile([128, SC], F32, tag="pv2")
                    for t in range(8):
                        nc.tensor.matmul(
                            pk2, lhsT=wk2_sb[:, t, 128 * hh:128 * (hh + 1)],
                            rhs=enc_sb[:, t, :],
                            start=(t == 0), stop=(t == 7))
                        nc.tensor.matmul(
                            pv2, lhsT=wv2_sb[:, t, 128 * hh:128 * (hh + 1)],
                            rhs=enc_sb[:, t, :],
                            start=(t == 0), stop=(t == 7))
                    nc.vector.tensor_copy(out=vT2_sb[:, hh, :], in_=pv2)
                    rms_epilogue(pk2, k2_sb[:, hh, :], 0, False, epool, qpsum2)
                for hh in range(HL):
                    for kt in range(4):
                        vtp2 = qpsum2.tile([128, 128], F32R, tag="ep_rot",
                                           name="vtp2")
                        nc.tensor.transpose(
                            vtp2, vT2_sb[:, hh, 128 * kt:128 * (kt + 1)],
                            eye_sb)
                        nc.vector.tensor_copy(
                            out=v2_sb[:, kt, 128 * hh:128 * (hh + 1)],
                            in_=vtp2)

                wq2_sb = wpool.tile([128, CT, HL * D], F32R, tag="wq2")
                nc.sync.dma_start(out=wq2_sb,
                                  in_=wq2.rearrange("(t p) m -> p t m", p=128))
                for ch in range(4):
                    pq = [qpsum.tile([128, 512], F32, tag=f"pq2_{j}", name=f"pq2_{j}")
                          for j in range(2)]
                    for ct in range(CT):
                        st, sp = (ct == 0), (ct == CT - 1)
                        xt = wpool.tile([128, 512], F32R, tag="xt2", bufs=4)
                        nc.sync.dma_start(out=xt,
                                          in_=agv[:, ct // 4, ct % 4, 2 * ch:2 * ch + 2, :])
                        for hh in range(HL):
                            nc.tensor.matmul(
                                pq[hh],
                                lhsT=wq2_sb[:, ct, 128 * hh:128 * (hh + 1)],
                                rhs=xt, start=st, stop=sp)
                    for hh in range(HL):
                        rms_epilogue(pq[hh],
                                     q2_sb[:, hh, 512 * ch:512 * (ch + 1)],
                                     ch, False, epool, qpsum2)

            with tc.tile_pool(name="sdpa2", bufs=2) as spool, \
                 tc.tile_pool(name="sdpa2ps", bufs=2, space="PSUM") as spsum:
                wo2_sb = spool.tile([128, 2, C], F32R, tag="wo2", bufs=1)
                nc.sync.dma_start(out=wo2_sb,
                                  in_=wo2.rearrange("(t p) m -> p t m", p=128))
                for ch in range(4):
                    a0 = spool.tile([D, 512], F32R, tag="a20")
                    a1 = spool.tile([D, 512], F32R, tag="a21")
                    sdpa(q2_sb, k2_sb, v2_sb, 4, 0, ch, a0, spool, spsum)
                    sdpa(q2_sb, k2_sb, v2_sb, 4, 1, ch, a1, spool, spsum)
                    wo_phase(a0, a1, wo2_sb, ch, 1, spool, spsum)

        reduce_and_residual(1)

        # ======================= sublayer 2: feed forward ==================
        with tc.tile_pool(name="mod2", bufs=2) as mpool, \
             tc.tile_pool(name="mod2ps", bufs=2, space="PSUM") as mpsum:
            modulate(2, mpool, mpsum)

        with tc.tile_pool(name="ff", bufs=1) as fpool:
            g_sb = fpool.tile([128, 8, S], F32R, tag="g")
            agv = agv_of(2)
            with tc.tile_pool(name="ff1", bufs=1) as wpool, \
                 tc.tile_pool(name="ff1ps", bufs=2, space="PSUM") as fpsum:
                w1_sb = wpool.tile([128, CT, FFL], F32R, tag="w1")
                nc.sync.dma_start(out=w1_sb,
                                  in_=w1f.rearrange("(t p) m -> p t m", p=128))
                for ch in range(4):
                    xncol = wpool.tile([128, CT, 512], F32R, tag="xncolf")
                    for ct in range(CT):
                        nc.sync.dma_start(out=xncol[:, ct, :],
                                          in_=agv[:, ct // 4, ct % 4, 2 * ch:2 * ch + 2, :])
                    for ft in range(8):
                        gps = fpsum.tile([128, 512], F32, tag="gps")
                        for ct in range(CT):
                            nc.tensor.matmul(
                                gps, lhsT=w1_sb[:, ct, 128 * ft:128 * (ft + 1)],
                                rhs=xncol[:, ct, :],
                                start=(ct == 0), stop=(ct == CT - 1))
                        nc.scalar.activation(
                            out=g_sb[:, ft, 512 * ch:512 * (ch + 1)],
                            in_=gps, func=AF.Gelu)  # xncol reused by 8 ft

            with tc.tile_pool(name="ff2", bufs=2) as wpool, \
                 tc.tile_pool(name="ff2ps", bufs=3, space="PSUM") as fpsum:
                w2v = w2f.rearrange("(t p) (cc n) -> p t cc n", p=128, n=512)
                for cc in range(4):
                    w2c = wpool.tile([128, 8, 512], F32R, tag="w2c")
                    nc.sync.dma_start(out=w2c, in_=w2v[:, :, cc, :])
                    for st in range(CT):
                        yps = fpsum.tile([128, 512], F32, tag="yps")
                        for ft in range(8):
                            nc.tensor.matmul(
                                yps, lhsT=g_sb[:, ft, 128 * st:128 * (st + 1)],
                                rhs=w2c[:, ft, :],
                                start=(ft == 0), stop=(ft == 7))
                        ye = wpool.tile([128, 512], F32, tag="ye")
                        nc.vector.tensor_copy(out=ye, in_=yps)
                        rsv = rs_in[2][cc].rearrange("(st p) c -> p st c",
                                                     p=128)
                        nc.sync.dma_start(out=rsv[:, st, :], in_=ye)

        reduce_and_residual(2)

        nc.sync.dma_start(out=h_out.rearrange("(u p) c -> p u c", p=128),
                          in_=h_sb)

        ctx_pool.__exit__(None, None, None)

    nc.compile()
    return nc


_NC_CACHE = None


def _get_nc():
    global _NC_CACHE
    if _NC_CACHE is None:
        nc = bacc.Bacc("TRN2", target_bir_lowering=False, debug=False,
                       num_devices=NCORES)
        _NC_CACHE = _build(nc)
    return _NC_CACHE


def kernel(**inputs) -> np.ndarray:
    h = np.asarray(inputs["hidden_states"], np.float32)[0]      # [S, C]
    enc = np.asarray(inputs["encoder_hidden_states"], np.float32)[0]
    emb = np.asarray(inputs["embedded_timestep"], np.float32)[0]  # [C]
    temb = np.asarray(inputs["temb"], np.float32)                # [1, 3C]
    cos = np.asarray(inputs["rope_cos"], np.float32)             # [S, D]
    sin = np.asarray(inputs["rope_sin"], np.float32)

    # rms-norm affine weights are ones per the module config; verify.
    for k in ("attn1_qn", "attn1_kn", "attn2_qn", "attn2_kn"):
        assert np.allclose(np.asarray(inputs[k]), 1.0), f"{k} != ones"

    rot = np.zeros((D, D), np.float32)  # rot_out = rot_m.T @ q
    for d in range(D // 2):
        rot[64 + d, d] = -1.0   # out[d] = -q[64+d]
        rot[d, 64 + d] = 1.0    # out[64+d] = q[d]

    r = round_fp32r
    common = {
        "encT": r(enc.T), "aw1": r(np.stack([inputs["a1_w1"],
                                             inputs["a2_w1"],
                                             inputs["a3_w1"]])),
        "aw2": r(np.stack([inputs["a1_w2"], inputs["a2_w2"],
                           inputs["a3_w2"]])),
        "emb_t": np.ascontiguousarray(emb.reshape(CT, 128).T),
        "temb_r": np.ascontiguousarray(temb),
        "cosT": np.ascontiguousarray(cos.T), "sinT": np.ascontiguousarray(sin.T),
        "rotm": r(rot), "eye": r(np.eye(128, dtype=np.float32)),
        "onec": np.ones((128, 1), np.float32),
    }
    in_maps = []
    for i in range(NCORES):
        hs = slice(HL * D * i, HL * D * (i + 1))   # head-dim slice (256)
        fs = slice(FFL * i, FFL * (i + 1))         # ff slice (1024)
        m = dict(common)
        m["h_s"] = r(h[SL * i:SL * (i + 1), :])
        m["wq1"] = r(np.asarray(inputs["attn1_wq"])[:, hs])
        m["wk1"] = r(np.asarray(inputs["attn1_wk"])[:, hs])
        m["wv1"] = r(np.asarray(inputs["attn1_wv"])[:, hs])
        m["wo1"] = r(np.asarray(inputs["attn1_wo"])[hs, :])
        m["wq2"] = r(np.asarray(inputs["attn2_wq"])[:, hs])
        m["wk2"] = r(np.asarray(inputs["attn2_wk"])[:, hs])
        m["wv2"] = r(np.asarray(inputs["attn2_wv"])[:, hs])
        m["wo2"] = r(np.asarray(inputs["attn2_wo"])[hs, :])
        m["w1f"] = r(np.asarray(inputs["ff_w1"])[:, fs])
        m["w2f"] = r(np.asarray(inputs["ff_w2"])[fs, :])
        in_maps.append({k: np.ascontiguousarray(v, np.float32)
                        for k, v in m.items()})

    nc = _get_nc()
    res = run_bass_kernel_spmd(nc, in_maps, core_ids=list(range(NCORES)))
    out = np.concatenate([res.results[i]["h_out"] for i in range(NCORES)],
                         axis=0)
    return out.reshape(1, S, C).astype(np.float32)


if __name__ == "__main__":
    _get_nc()
    print("build + compile OK")



# revision 2
# speedup vs baseline: 1004.5415x; 1004.5415x over previous
"""CosmosTransformerBlock on 8 Trainium2 NeuronCores — sequence-parallel.

Strategy v2 (sequence-parallel, replicated weights, bf16 matmuls):
- Each core owns SL=256 tokens of the residual stream; LayerNorm/AdaLN
  modulation, QKV, attention queries, wo, and the whole FF run locally on
  those tokens with full (replicated) weight matrices — identical FLOPs to
  tensor-parallel, but attn2 and FF need NO collectives at all.
- Self-attention needs full-sequence K/V: one fused AllGather of the local
  [K^T; V] buffer (bf16). Cross-attention K/V is computed head-sharded from
  the encoder (2 heads/core) and AllGather'd early (overlaps sublayer 0).
- AdaLN lora-up matmul is sharded 768 cols/core; the tiny e vectors are
  AllGather'd at kernel start.
- All matmuls run in bf16 (1 cyc/row on PE) with fp32 PSUM accumulation;
  residual h, LN stats, softmax normalization stay fp32.
"""

import sys

import numpy as np

try:
    import concourse.bass as bass
except ImportError:  # pragma: no cover
    sys.path.insert(0, "/opt/trn_rl_repo")
    import concourse.bass as bass

import ml_dtypes
import concourse.mybir as mybir
import concourse.tile as tile
from concourse import bacc
from concourse.bass_utils import run_bass_kernel_spmd

F32 = mybir.dt.float32
F32R = mybir.dt.float32r
BF16 = mybir.dt.bfloat16
AF = mybir.ActivationFunctionType
ALU = mybir.AluOpType

NCORES = 8
S, C, D, H = 2048, 2048, 128, 16
SL = S // NCORES      # 256 tokens per core
CT = C // 128         # 16 feature tiles
SC, CROSS, LORA, FF = 512, 1024, 256, 8192
FT = FF // 128        # 64 ff tiles
EPS = 1e-6
ISQD = float(D) ** -0.5

KV1K = H * D * SL     # 524288: K^T elems in kv1_in
KV1SZ = 2 * KV1K
KV2K = 2 * D * SC     # 131072: local-2-head K2^T elems in kv2_in
KV2SZ = 2 * KV2K
RG = [list(range(NCORES))]


def round_fp32r(a: np.ndarray) -> np.ndarray:
    b = np.ascontiguousarray(a, dtype=np.float32).view(np.uint32)
    lsb = (b >> np.uint32(12)) & np.uint32(1)
    r = (b + np.uint32(0x7FF) + lsb) & np.uint32(0xFFFFF000)
    return r.view(np.float32)


def _build(nc: bacc.Bacc):
    dram = lambda n, s, d: nc.dram_tensor(n, s, d, kind="ExternalInput").ap()

    h_in = dram("h_s", [SL, C], F32)
    wq1 = dram("wq1", [C, C], BF16)
    wk1 = dram("wk1", [C, C], BF16)
    wv1 = dram("wv1", [C, C], BF16)
    wo1 = dram("wo1", [C, C], BF16)
    wq2 = dram("wq2", [C, C], BF16)
    wk2 = dram("wk2", [CROSS, C], BF16)
    wv2 = dram("wv2", [CROSS, C], BF16)
    wo2 = dram("wo2", [C, C], BF16)
    w1f = dram("w1f", [C, FF], BF16)
    w2f = dram("w2f", [FF, C], BF16)
    encT = dram("encT", [CROSS, SC], BF16)
    aw1 = dram("aw1", [3, C, LORA], BF16)
    aw2l = dram("aw2l", [3, LORA, 768], BF16)
    temb_l = dram("temb_l", [3, 256], F32)
    emb_t = dram("emb_t", [128, CT], F32)
    cosT = dram("cosT", [D, SL], F32)
    sinT = dram("sinT", [D, SL], F32)
    rotm = dram("rotm", [D, D], F32R)
    eye = dram("eye", [128, 128], F32R)
    onec = dram("onec", [128, 1], F32R)
    onecb = dram("onecb", [128, 1], BF16)

    h_out = nc.dram_tensor("h_out", [SL, C], F32, kind="ExternalOutput").ap()

    ydr = nc.dram_tensor("ydr", [3, 256], BF16).ap()
    e_in = nc.dram_tensor("e_in", [9, 256], F32).ap()
    e_out = nc.dram_tensor("e_out", [NCORES * 9, 256], F32,
                           addr_space="Shared").ap()
    kv2d = nc.dram_tensor("kv2d", [2 * 128 * H * SC], BF16).ap()
    kv1_in = nc.dram_tensor("kv1_in", [KV1SZ], BF16).ap()
    kv1_out = nc.dram_tensor("kv1_out", [NCORES * KV1SZ], BF16,
                             addr_space="Shared").ap()

    with tile.TileContext(nc) as tc:
        ctx_pool = tc.tile_pool(name="persist", bufs=1)
        persist = ctx_pool.__enter__()

        h_sb = persist.tile([128, 2, C], F32, tag="h")
        nc.sync.dma_start(out=h_sb, in_=h_in.rearrange("(u p) c -> p u c",
                                                       p=128))
        eye_sb = persist.tile([128, 128], F32R, tag="eye")
        nc.sync.dma_start(out=eye_sb, in_=eye)
        onec_sb = persist.tile([128, 1], F32R, tag="onec")
        nc.sync.dma_start(out=onec_sb, in_=onec)
        onecb_sb = persist.tile([128, 1], BF16, tag="onecb")
        nc.sync.dma_start(out=onecb_sb, in_=onecb)
        rotm_sb = persist.tile([D, D], F32R, tag="rotm")
        nc.sync.dma_start(out=rotm_sb, in_=rotm)
        emb_sb = persist.tile([128, CT], F32, tag="emb")
        nc.sync.dma_start(out=emb_sb, in_=emb_t)
        eps_sb = persist.tile([128, 1], F32, tag="eps")
        nc.vector.memset(eps_sb, EPS)
        # cos/sin tiled x2 for 2-head batched rope epilogues
        cos2_sb = persist.tile([D, 2, SL], F32, tag="cos2")
        nc.sync.dma_start(out=cos2_sb, in_=bass.AP(
            tensor=cosT.tensor, offset=0, ap=[[SL, 128], [0, 2], [1, SL]]))
        sin2_sb = persist.tile([D, 2, SL], F32, tag="sin2")
        nc.sync.dma_start(out=sin2_sb, in_=bass.AP(
            tensor=sinT.tensor, offset=0, ap=[[SL, 128], [0, 2], [1, SL]]))
        gateB = persist.tile([128, C], F32, tag="gateB")
        q_sb = persist.tile([128, H, SL], BF16, tag="q")
        xnT_sb = persist.tile([128, CT, SL], BF16, tag="xnT")
        a_sb = persist.tile([128, H, SL], BF16, tag="a")

        # ---------------- AdaLN (sharded 768 cols/core) + e AllGather -------
        # All three sublayers' lora-down products y_L are computed first and
        # round-trip DRAM once; then the lora-up slices + temb -> e AllGather.
        with tc.tile_pool(name="adaln", bufs=3) as apool, \
             tc.tile_pool(name="adaln_ps", bufs=3, space="PSUM") as apsum:
            silu_sb = apool.tile([128, CT], BF16, tag="silu", bufs=1)
            nc.scalar.activation(out=silu_sb, in_=emb_sb, func=AF.Silu)
            temb_sb = apool.tile([1, 3, 256], F32, tag="tembl", bufs=1)
            nc.sync.dma_start(out=temb_sb, in_=bass.AP(
                tensor=temb_l.tensor, offset=0,
                ap=[[768, 1], [256, 3], [1, 256]]))
            e_loc = apool.tile([1, 9, 256], F32, tag="eloc", bufs=1)
            yall = apool.tile([1, 3, LORA], BF16, tag="yall", bufs=1)
            for L in range(3):
                aw1_sb = apool.tile([128, CT, LORA], BF16, tag="aw1")
                nc.sync.dma_start(
                    out=aw1_sb, in_=aw1[L].rearrange("(t p) m -> p t m",
                                                     p=128))
                y_ps = apsum.tile([1, LORA], F32, tag="yps")
                for ct in range(CT):
                    nc.tensor.matmul(y_ps, lhsT=silu_sb[:, ct:ct + 1],
                                     rhs=aw1_sb[:, ct, :],
                                     start=(ct == 0), stop=(ct == CT - 1))
                nc.scalar.copy(out=yall[:, L, :], in_=y_ps)
            nc.sync.dma_start(out=bass.AP(
                tensor=ydr.tensor, offset=0,
                ap=[[768, 1], [256, 3], [1, 256]]), in_=yall)
            y2 = apool.tile([128, 3, 2], BF16, tag="y2", bufs=1)
            nc.sync.dma_start(out=y2, in_=bass.AP(
                tensor=ydr.tensor, offset=0,
                ap=[[1, 128], [256, 3], [128, 2]]))
            for L in range(3):
                aw2_sb = apool.tile([128, 2, 3, 256], BF16, tag="aw2")
                nc.sync.dma_start(
                    out=aw2_sb,
                    in_=aw2l[L].rearrange("(t p) (g n) -> p t g n",
                                          p=128, n=256))
                for g3 in range(3):
                    e_ps = apsum.tile([1, 256], F32, tag="eps2")
                    nc.tensor.matmul(e_ps, lhsT=y2[:, L, 0:1],
                                     rhs=aw2_sb[:, 0, g3, :],
                                     start=True, stop=False)
                    nc.tensor.matmul(e_ps, lhsT=y2[:, L, 1:2],
                                     rhs=aw2_sb[:, 1, g3, :],
                                     start=False, stop=True)
                    nc.vector.tensor_add(out=e_loc[:, 3 * L + g3, :],
                                         in0=e_ps,
                                         in1=temb_sb[:, g3, :])
            nc.sync.dma_start(out=bass.AP(
                tensor=e_in.tensor, offset=0,
                ap=[[2304, 1], [256, 9], [1, 256]]), in_=e_loc)
        nc.gpsimd.collective_compute(
            "AllGather", ALU.bypass, replica_groups=RG,
            ins=[e_in], outs=[e_out])

        def _ebcast(L, t):
            # [128, 8, 256] broadcast view of e_out for (sublayer L, type t)
            return bass.AP(tensor=e_out.tensor, offset=(3 * L + t) * 256,
                           ap=[[0, 128], [9 * 256, 8], [1, 256]])

        # ---------------- modulate: LN + transpose + feature-major mod ------
        # LN and the transposes do NOT wait for the e AllGather; scale/shift
        # are applied per-partition (feature) on the transposed tiles.
        def modulate(L, mpool, mpsum):
            shT = mpool.tile([128, 2, 8], F32, tag="shT", bufs=1)
            opT = mpool.tile([128, 2, 8], F32, tag="opT", bufs=1)
            for par in range(2):
                nc.sync.dma_start(out=shT[:, par, :], in_=bass.AP(
                    tensor=e_out.tensor, offset=3 * L * 256 + par * 128,
                    ap=[[1, 128], [2304, 8]]))
                nc.sync.dma_start(out=opT[:, par, :], in_=bass.AP(
                    tensor=e_out.tensor, offset=(3 * L + 1) * 256 + par * 128,
                    ap=[[1, 128], [2304, 8]]))
            nc.vector.tensor_scalar_add(out=opT, in0=opT, scalar1=1.0)
            nc.sync.dma_start(out=gateB, in_=_ebcast(L, 2))
            for u in range(2):
                x = h_sb[:, u, :]
                stats = mpool.tile([128, 4, 6], F32, tag="stats")
                xv = x.rearrange("p (g n) -> p g n", n=512)
                for g in range(4):
                    nc.vector.bn_stats(out=stats[:, g, :], in_=xv[:, g, :])
                mv = mpool.tile([128, 2], F32, tag="mv")
                nc.vector.bn_aggr(out=mv, in_=stats)
                sd = mpool.tile([128, 1], F32, tag="sd")
                nc.scalar.activation(out=sd, in_=mv[:, 1:2], func=AF.Sqrt,
                                     bias=eps_sb)
                rstd = mpool.tile([128, 1], F32, tag="rstd")
                nc.vector.reciprocal(out=rstd, in_=sd)
                t1 = mpool.tile([128, C], F32R, tag="mod_t1",
                                name=f"t1_{u}")
                nc.vector.tensor_scalar(out=t1, in0=x, scalar1=mv[:, 0:1],
                                        scalar2=rstd, op0=ALU.subtract,
                                        op1=ALU.mult)
                for ct in range(CT):
                    tp = mpsum.tile([128, 128], F32R, tag="tpsum")
                    nc.tensor.transpose(
                        tp, t1[:, 128 * ct:128 * (ct + 1)], eye_sb)
                    nc.vector.tensor_scalar(
                        out=xnT_sb[:, ct, 128 * u:128 * (u + 1)], in0=tp,
                        scalar1=opT[:, ct % 2, ct // 2:ct // 2 + 1],
                        scalar2=shT[:, ct % 2, ct // 2:ct // 2 + 1],
                        op0=ALU.mult, op1=ALU.add)

        # batched rms(+rope) epilogue: src sbuf f32r [128, 2, 256] (2 heads)
        def rms_ep(src, dst, rope, pool, psum):
            sq = pool.tile([128, 2 * SL], F32R, tag="ep_sq")
            nc.scalar.activation(out=sq, in_=src, func=AF.Square)
            ss = psum.tile([1, 2 * SL], F32, tag="ep_ss")
            nc.tensor.matmul(ss, lhsT=onec_sb, rhs=sq, start=True, stop=True)
            sd = pool.tile([1, 2 * SL], F32, tag="ep_sd")
            nc.scalar.activation(out=sd, in_=ss, func=AF.Sqrt, scale=1.0 / D,
                                 bias=eps_sb[0:1, :])
            rc = pool.tile([1, 2 * SL], F32, tag="ep_rc")
            nc.vector.reciprocal(out=rc, in_=sd)
            rb = pool.tile([128, 2 * SL], F32, tag="ep_rb")
            nc.gpsimd.partition_broadcast(out_ap=rb, in_ap=rc)
            if rope:
                rot = psum.tile([128, 2 * SL], F32, tag="ep_rot")
                nc.tensor.matmul(rot, lhsT=rotm_sb, rhs=src,
                                 start=True, stop=True)
                t1 = pool.tile([128, 2 * SL], F32, tag="ep_t1")
                nc.vector.tensor_mul(out=t1, in0=src, in1=cos2_sb)
                t2 = pool.tile([128, 2 * SL], F32, tag="ep_t2")
                nc.vector.tensor_mul(out=t2, in0=rot, in1=sin2_sb)
                nc.vector.tensor_add(out=t1, in0=t1, in1=t2)
                nc.vector.tensor_mul(out=dst, in0=t1, in1=rb)
            else:
                nc.vector.tensor_mul(out=dst, in0=src, in1=rb)

        # wo + gated residual (shared by attn1/attn2); first chunks may
        # be prefetched into `pre` while the preceding phase runs
        def wo_prefetch(wX, pool, n=2):
            wv_ = wX.rearrange("(t p) (cc m) -> p cc t m", p=128, m=512)
            pre = []
            for cc in range(n):
                wo_sb = pool.tile([128, CT, 512], BF16, tag="wos")
                nc.sync.dma_start(out=wo_sb, in_=wv_[:, cc])
                pre.append(wo_sb)
            return wv_, pre

        def wo_residual(wv_, pool, pre):
            with tc.tile_pool(name="wops", bufs=2, space="PSUM") as psum:
                for cc in range(4):
                    if cc < len(pre):
                        wo_sb = pre[cc]
                    else:
                        wo_sb = pool.tile([128, CT, 512], BF16, tag="wos")
                        nc.sync.dma_start(out=wo_sb, in_=wv_[:, cc])
                    for u in range(2):
                        ops = psum.tile([128, 512], F32, tag="wops")
                        for hh in range(H):
                            nc.tensor.matmul(
                                ops,
                                lhsT=a_sb[:, hh, 128 * u:128 * (u + 1)],
                                rhs=wo_sb[:, hh, :],
                                start=(hh == 0), stop=(hh == H - 1))
                        gt = pool.tile([128, 512], F32, tag="wogt")
                        nc.vector.tensor_mul(
                            out=gt, in0=ops,
                            in1=gateB[:, 512 * cc:512 * (cc + 1)])
                        nc.vector.tensor_add(
                            out=h_sb[:, u, 512 * cc:512 * (cc + 1)],
                            in0=h_sb[:, u, 512 * cc:512 * (cc + 1)], in1=gt)

        # ======================= sublayer 0: self attention ================
        with tc.tile_pool(name="mod0", bufs=2) as mpool, \
             tc.tile_pool(name="mod0ps", bufs=2, space="PSUM") as mpsum:
            modulate(0, mpool, mpsum)

        # K/V in 4 head-groups of 4, written to k1_in/v1_in, then AllGathers
        wq1v = wq1.rearrange("(t p) (g m) -> p g t m", p=128, m=512)
        wk1v = wk1.rearrange("(t p) (g m) -> p g t m", p=128, m=512)
        wv1v = wv1.rearrange("(t p) (g m) -> p g t m", p=128, m=512)
        with tc.tile_pool(name="kv1w", bufs=2) as wpool, \
             tc.tile_pool(name="kv1e", bufs=1) as epool, \
             tc.tile_pool(name="kv1ps", bufs=1, space="PSUM") as qpsum, \
             tc.tile_pool(name="kv1ps2", bufs=1, space="PSUM") as qpsum2:
            for g in range(4):
                wk_sb = wpool.tile([128, CT, 512], BF16, tag="wk")
                nc.sync.dma_start(out=wk_sb, in_=wk1v[:, g])
                wv_sb = wpool.tile([128, CT, 512], BF16, tag="wv")
                nc.sync.dma_start(out=wv_sb, in_=wv1v[:, g])
                pk = qpsum.tile([128, 4, SL], F32, tag="pk")
                pv = qpsum.tile([128, 2, 512], F32, tag="pv")
                for ct in range(CT):
                    st, sp = (ct == 0), (ct == CT - 1)
                    for j in range(4):
                        nc.tensor.matmul(
                            pk[:, j, :],
                            lhsT=wk_sb[:, ct, 128 * j:128 * (j + 1)],
                            rhs=xnT_sb[:, ct, :], start=st, stop=sp)
                    for u in range(2):
                        nc.tensor.matmul(
                            pv[:, u, :],
                            lhsT=xnT_sb[:, ct, 128 * u:128 * (u + 1)],
                            rhs=wv_sb[:, ct, :], start=st, stop=sp)
                kf = epool.tile([128, 4, SL], F32R, tag="kf")
                nc.vector.tensor_copy(out=kf, in_=pk)
                ve = epool.tile([128, 2, 512], BF16, tag="ve")
                nc.vector.tensor_copy(out=ve, in_=pv)
                nc.sync.dma_start(
                    out=bass.AP(tensor=kv1_in.tensor,
                                offset=KV1K + 512 * g,
                                ap=[[C, 128], [128 * C, 2], [1, 512]]),
                    in_=ve)
                for jj in range(2):
                    ke = epool.tile([128, 2, SL], BF16, tag="ke")
                    rms_ep(kf[:, 2 * jj:2 * jj + 2, :], ke, True,
                           epool, qpsum2)
                    nc.sync.dma_start(
                        out=bass.AP(
                            tensor=kv1_in.tensor,
                            offset=(4 * g + 2 * jj) * D * SL,
                            ap=[[SL, 128], [D * SL, 2], [1, SL]]),
                        in_=ke)
        nc.gpsimd.collective_compute(
            "AllGather", ALU.bypass, replica_groups=RG,
            ins=[kv1_in], outs=[kv1_out])

        # ---- during the AllGather window: Q (rms+rope) and enc K2/V2 ------
        with tc.tile_pool(name="q1w", bufs=2) as wpool, \
             tc.tile_pool(name="q1e", bufs=1) as epool, \
             tc.tile_pool(name="q1ps", bufs=1, space="PSUM") as qpsum, \
             tc.tile_pool(name="q1ps2", bufs=1, space="PSUM") as qpsum2:
            for g in range(4):
                wq_sb = wpool.tile([128, CT, 512], BF16, tag="wq")
                nc.scalar.dma_start(out=wq_sb, in_=wq1v[:, g])
                pq = qpsum.tile([128, 4, SL], F32, tag="pq")
                for ct in range(CT):
                    st, sp = (ct == 0), (ct == CT - 1)
                    for j in range(4):
                        nc.tensor.matmul(
                            pq[:, j, :],
                            lhsT=wq_sb[:, ct, 128 * j:128 * (j + 1)],
                            rhs=xnT_sb[:, ct, :], start=st, stop=sp)
                qf = epool.tile([128, 4, SL], F32R, tag="qf")
                nc.vector.tensor_copy(out=qf, in_=pq)
                for jj in range(2):
                    h0 = 4 * g + 2 * jj
                    rms_ep(qf[:, 2 * jj:2 * jj + 2, :],
                           q_sb[:, h0:h0 + 2, :], True, epool, qpsum2)

        # replicated cross-attn K2^T/V2 from the encoder (all 16 heads);
        # results are spilled to DRAM and reloaded at sdpa2 to keep SBUF free
        wk2v = wk2.rearrange("(t p) (g m) -> p g t m", p=128, m=256)
        wv2v = wv2.rearrange("(t p) (g m) -> p g t m", p=128, m=256)
        with tc.tile_pool(name="enckv", bufs=2) as kpool, \
             tc.tile_pool(name="enckvps", bufs=1, space="PSUM") as kpsum, \
             tc.tile_pool(name="enckvps2", bufs=1, space="PSUM") as kpsum2:
            k2_sb = kpool.tile([128, H, SC], BF16, tag="k2", bufs=1)
            v2_sb = kpool.tile([128, 4, H, D], BF16, tag="v2", bufs=1)
            enc_sb = kpool.tile([128, 8, SC], BF16, tag="enc", bufs=1)
            nc.scalar.dma_start(out=enc_sb,
                              in_=encT.rearrange("(t p) s -> p t s", p=128))
            for g in range(8):  # pairs of heads
                wk2_sb = kpool.tile([128, 8, 256], BF16, tag="wk2")
                nc.scalar.dma_start(out=wk2_sb, in_=wk2v[:, g])
                wv2_sb = kpool.tile([128, 8, 256], BF16, tag="wv2")
                nc.scalar.dma_start(out=wv2_sb, in_=wv2v[:, g])
                pk2 = kpsum.tile([128, 2, SC], F32, tag="pk2")
                pv2 = kpsum.tile([128, 4, 256], F32, tag="pv2")
                for t in range(8):
                    st, sp = (t == 0), (t == 7)
                    for j in range(2):
                        nc.tensor.matmul(
                            pk2[:, j, :],
                            lhsT=wk2_sb[:, t, 128 * j:128 * (j + 1)],
                            rhs=enc_sb[:, t, :], start=st, stop=sp)
                    for tt in range(4):
                        nc.tensor.matmul(
                            pv2[:, tt, :],
                            lhsT=enc_sb[:, t, 128 * tt:128 * (tt + 1)],
                            rhs=wv2_sb[:, t, :], start=st, stop=sp)
                kf2 = kpool.tile([128, 2, SC], F32R, tag="kf2")
                nc.vector.tensor_copy(out=kf2, in_=pk2)
                # batched rms over head dim (partition), no rope
                sq = kpool.tile([128, 2, SC], F32R, tag="k2sq")
                nc.scalar.activation(out=sq, in_=kf2, func=AF.Square)
                ss = kpsum2.tile([1, 2, SC], F32, tag="k2ss")
                for j in range(2):
                    nc.tensor.matmul(ss[:, j, :], lhsT=onec_sb,
                                     rhs=sq[:, j, :], start=True, stop=True)
                sd = kpool.tile([1, 2 * SC], F32, tag="k2sd")
                nc.scalar.activation(out=sd, in_=ss, func=AF.Sqrt,
                                     scale=1.0 / D, bias=eps_sb[0:1, :])
                rc = kpool.tile([1, 2 * SC], F32, tag="k2rc")
                nc.vector.reciprocal(out=rc, in_=sd)
                rb = kpool.tile([128, 2 * SC], F32, tag="k2rb")
                nc.gpsimd.partition_broadcast(out_ap=rb, in_ap=rc)
                nc.vector.tensor_mul(out=k2_sb[:, 2 * g:2 * g + 2, :],
                                     in0=kf2, in1=rb)
                nc.vector.tensor_copy(out=v2_sb[:, :, 2 * g:2 * g + 2, :],
                                      in_=pv2)
            nc.sync.dma_start(out=bass.AP(
                tensor=kv2d.tensor, offset=0,
                ap=[[H * SC, 128], [1, H * SC]]), in_=k2_sb)
            nc.sync.dma_start(out=bass.AP(
                tensor=kv2d.tensor, offset=128 * H * SC,
                ap=[[H * SC, 128], [1, H * SC]]), in_=v2_sb)

        # sdpa over 2 halves of 8 heads; 4-head interleaved softmax rounds.
        # PE emission is software-pipelined: scores(kt) are issued before
        # den/av(kt-1) so PE never stalls on the Exp of the current round.
        def sdpa1(spool, spsum, spsum2):
            for quarter in range(4):
                hb0 = 4 * quarter
                k_sb = spool.tile([128, 4, 8, SL], BF16, tag="ksb", bufs=2)
                v_sb = spool.tile([128, 16, 512], BF16, tag="vsb", bufs=2)
                for r in range(NCORES):
                    nc.sync.dma_start(out=k_sb[:, :, r, :], in_=bass.AP(
                        tensor=kv1_out.tensor,
                        offset=hb0 * D * SL + r * KV1SZ,
                        ap=[[SL, 128], [D * SL, 4], [1, SL]]))
                    nc.sync.dma_start(out=v_sb[:, 2 * r:2 * r + 2, :],
                                      in_=bass.AP(
                        tensor=kv1_out.tensor,
                        offset=KV1K + hb0 * D + r * KV1SZ,
                        ap=[[C, 128], [128 * C, 2], [1, 512]]))
                if True:
                    hb = 0
                    av = spsum.tile([128, 4, SL], F32, tag="av")
                    den = spsum.tile([1, 2, 2 * SL], F32, tag="den")

                    def sc_round(kt):
                        sps = spsum2.tile([128, 4, SL], F32, tag="sps",
                                          name=f"sps{kt}")
                        for j in range(4):
                            nc.tensor.matmul(
                                sps[:, j, :],
                                lhsT=k_sb[:, j, kt // 2,
                                          128 * (kt % 2):128 * (kt % 2) + 128],
                                rhs=q_sb[:, hb0 + j, :],
                                start=True, stop=True)
                        pt = spool.tile([128, 4, SL], BF16, tag="pt",
                                        name=f"pt{kt}")
                        nc.scalar.activation(out=pt, in_=sps, func=AF.Exp,
                                             scale=ISQD)
                        return pt

                    def av_round(kt, pt):
                        st, sp = (kt == 0), (kt == 15)
                        for dh in range(2):
                            nc.tensor.matmul(
                                den[:, dh, :], lhsT=onecb_sb,
                                rhs=pt[:, 2 * dh:2 * dh + 2, :],
                                start=st, stop=sp)
                        for j in range(4):
                            nc.tensor.matmul(
                                av[:, j, :],
                                lhsT=v_sb[:, kt, 128 * j:128 * (j + 1)],
                                rhs=pt[:, j, :], start=st, stop=sp)

                    prev = sc_round(0)
                    for kt in range(1, 16):
                        cur = sc_round(kt)
                        av_round(kt - 1, prev)
                        prev = cur
                    av_round(15, prev)
                    rc = spool.tile([1, 4 * SL], F32, tag="sd_rc")
                    nc.vector.reciprocal(out=rc, in_=den)
                    rb = spool.tile([128, 4 * SL], F32, tag="sd_rb")
                    nc.gpsimd.partition_broadcast(out_ap=rb, in_ap=rc)
                    nc.vector.tensor_mul(
                        out=a_sb[:, hb0:hb0 + 4, :],
                        in0=av, in1=rb)

        wo1_ctx = tc.tile_pool(name="wo1", bufs=2)
        wo1_pool = wo1_ctx.__enter__()
        wo1v, wo1_pre = wo_prefetch(wo1, wo1_pool)

        with tc.tile_pool(name="sdpa1", bufs=3) as spool, \
             tc.tile_pool(name="sdpa1ps", bufs=1, space="PSUM") as spsum, \
             tc.tile_pool(name="sdpa1ps2", bufs=2, space="PSUM") as spsum2:
            sdpa1(spool, spsum, spsum2)

        wo_residual(wo1v, wo1_pool, wo1_pre)
        wo1_ctx.__exit__(None, None, None)

        # ======================= sublayer 1: cross attention ===============
        with tc.tile_pool(name="mod1", bufs=2) as mpool, \
             tc.tile_pool(name="mod1ps", bufs=2, space="PSUM") as mpsum:
            modulate(1, mpool, mpsum)

        wq2v = wq2.rearrange("(t p) (g m) -> p g t m", p=128, m=512)
        with tc.tile_pool(name="q2w", bufs=2) as wpool, \
             tc.tile_pool(name="q2e", bufs=1) as epool, \
             tc.tile_pool(name="q2ps", bufs=1, space="PSUM") as qpsum, \
             tc.tile_pool(name="q2ps2", bufs=1, space="PSUM") as qpsum2:
            for g in range(4):
                wq_sb = wpool.tile([128, CT, 512], BF16, tag="wq2")
                nc.scalar.dma_start(out=wq_sb, in_=wq2v[:, g])
                pq = qpsum.tile([128, 4, SL], F32, tag="pq2")
                for ct in range(CT):
                    st, sp = (ct == 0), (ct == CT - 1)
                    for j in range(4):
                        nc.tensor.matmul(
                            pq[:, j, :],
                            lhsT=wq_sb[:, ct, 128 * j:128 * (j + 1)],
                            rhs=xnT_sb[:, ct, :], start=st, stop=sp)
                qf = epool.tile([128, 4, SL], F32R, tag="qf2")
                nc.vector.tensor_copy(out=qf, in_=pq)
                for jj in range(2):
                    h0 = 4 * g + 2 * jj
                    rms_ep(qf[:, 2 * jj:2 * jj + 2, :],
                           q_sb[:, h0:h0 + 2, :], False, epool, qpsum2)

        def sdpa2(spool, spsum, spsum2):
            k2_sb = spool.tile([128, H, SC], BF16, tag="k2r", bufs=1)
            nc.sync.dma_start(out=k2_sb, in_=bass.AP(
                tensor=kv2d.tensor, offset=0,
                ap=[[H * SC, 128], [1, H * SC]]))
            v2_sb = spool.tile([128, 4, H, D], BF16, tag="v2r", bufs=1)
            nc.sync.dma_start(out=v2_sb, in_=bass.AP(
                tensor=kv2d.tensor, offset=128 * H * SC,
                ap=[[H * SC, 128], [1, H * SC]]))
            for sg in range(4):
                hb = 4 * sg
                av = spsum.tile([128, 4, SL], F32, tag="av2")
                den = spsum.tile([1, 2, 2 * SL], F32, tag="den2")

                def sc_round(kt):
                    sps = spsum2.tile([128, 4, SL], F32, tag="sps2",
                                      name=f"sps2_{kt}")
                    for j in range(4):
                        nc.tensor.matmul(
                            sps[:, j, :],
                            lhsT=k2_sb[:, hb + j, 128 * kt:128 * (kt + 1)],
                            rhs=q_sb[:, hb + j, :], start=True, stop=True)
                    pt = spool.tile([128, 4, SL], BF16, tag="pt2",
                                    name=f"pt2_{kt}")
                    nc.scalar.activation(out=pt, in_=sps, func=AF.Exp,
                                         scale=ISQD)
                    return pt

                def av_round(kt, pt):
                    st, sp = (kt == 0), (kt == 3)
                    for dh in range(2):
                        nc.tensor.matmul(
                            den[:, dh, :], lhsT=onecb_sb,
                            rhs=pt[:, 2 * dh:2 * dh + 2, :],
                            start=st, stop=sp)
                    for j in range(4):
                        nc.tensor.matmul(
                            av[:, j, :],
                            lhsT=v2_sb[:, kt, hb + j, :],
                            rhs=pt[:, j, :], start=st, stop=sp)

                prev = sc_round(0)
                for kt in range(1, 4):
                    cur = sc_round(kt)
                    av_round(kt - 1, prev)
                    prev = cur
                av_round(3, prev)
                rc = spool.tile([1, 4 * SL], F32, tag="sd2_rc")
                nc.vector.reciprocal(out=rc, in_=den)
                rb = spool.tile([128, 4 * SL], F32, tag="sd2_rb")
                nc.gpsimd.partition_broadcast(out_ap=rb, in_ap=rc)
                nc.vector.tensor_mul(out=a_sb[:, hb:hb + 4, :],
                                     in0=av, in1=rb)

        w1v = w1f.rearrange("(t p) (fg m) -> p fg t m", p=128, m=512)
        ff1_ctx = tc.tile_pool(name="ff1", bufs=2)
        ff1_pool = ff1_ctx.__enter__()
        wo2_ctx = tc.tile_pool(name="wo2", bufs=2)
        wo2_pool = wo2_ctx.__enter__()
        wo2v, wo2_pre = wo_prefetch(wo2, wo2_pool)
        w1_pre = []
        for fg in range(2):
            w1_sb = ff1_pool.tile([128, CT, 512], BF16, tag="w1")
            nc.sync.dma_start(out=w1_sb, in_=w1v[:, fg])
            w1_pre.append(w1_sb)

        with tc.tile_pool(name="sdpa2", bufs=3) as spool, \
             tc.tile_pool(name="sdpa2ps", bufs=1, space="PSUM") as spsum, \
             tc.tile_pool(name="sdpa2ps2", bufs=2, space="PSUM") as spsum2:
            sdpa2(spool, spsum, spsum2)

        wo_residual(wo2v, wo2_pool, wo2_pre)
        wo2_ctx.__exit__(None, None, None)

        # ======================= sublayer 2: feed forward ==================
        with tc.tile_pool(name="mod2", bufs=2) as mpool, \
             tc.tile_pool(name="mod2ps", bufs=2, space="PSUM") as mpsum:
            modulate(2, mpool, mpsum)

        w2v = w2f.rearrange("(f p) (cc m) -> p cc f m", p=128, m=256)
        with tc.tile_pool(name="ffg", bufs=1) as gpool:
            g_sb = gpool.tile([128, FT, SL], BF16, tag="g")
            with tc.tile_pool(name="ff1ps", bufs=4, space="PSUM") as fpsum:
                for fg in range(16):
                    if fg < len(w1_pre):
                        w1_sb = w1_pre[fg]
                    else:
                        w1_sb = ff1_pool.tile([128, CT, 512], BF16, tag="w1")
                        nc.sync.dma_start(out=w1_sb, in_=w1v[:, fg])
                    for ft in range(4):
                        gps = fpsum.tile([128, SL], F32, tag="gps")
                        for ct in range(CT):
                            nc.tensor.matmul(
                                gps,
                                lhsT=w1_sb[:, ct, 128 * ft:128 * (ft + 1)],
                                rhs=xnT_sb[:, ct, :],
                                start=(ct == 0), stop=(ct == CT - 1))
                        nc.scalar.activation(
                            out=g_sb[:, 4 * fg + ft, :], in_=gps,
                            func=AF.Gelu)
            with tc.tile_pool(name="ff2", bufs=2) as wpool, \
                 tc.tile_pool(name="ff2ps", bufs=2, space="PSUM") as fpsum:
                for cc in range(8):
                    w2_sb = wpool.tile([128, FT, 256], BF16, tag="w2")
                    nc.sync.dma_start(out=w2_sb, in_=w2v[:, cc])
                    for u in range(2):
                        yps = fpsum.tile([128, 256], F32, tag="yps")
                        for f in range(FT):
                            nc.tensor.matmul(
                                yps,
                                lhsT=g_sb[:, f, 128 * u:128 * (u + 1)],
                                rhs=w2_sb[:, f, :],
                                start=(f == 0), stop=(f == FT - 1))
                        gt = wpool.tile([128, 256], F32, tag="ffgt")
                        nc.vector.tensor_mul(
                            out=gt, in0=yps,
                            in1=gateB[:, 256 * cc:256 * (cc + 1)])
                        nc.vector.tensor_add(
                            out=h_sb[:, u, 256 * cc:256 * (cc + 1)],
                            in0=h_sb[:, u, 256 * cc:256 * (cc + 1)], in1=gt)

        ff1_ctx.__exit__(None, None, None)

        nc.sync.dma_start(out=h_out.rearrange("(u p) c -> p u c", p=128),
                          in_=h_sb)

        ctx_pool.__exit__(None, None, None)

    nc.compile()
    return nc


_NC_CACHE = None


def _get_nc():
    global _NC_CACHE
    if _NC_CACHE is None:
        nc = bacc.Bacc("TRN2", target_bir_lowering=False, debug=False,
                       num_devices=NCORES)
        _NC_CACHE = _build(nc)
    return _NC_CACHE


def _bf(a):
    return np.ascontiguousarray(np.asarray(a, np.float32)).astype(
        ml_dtypes.bfloat16)


def kernel(**inputs) -> np.ndarray:
    h = np.asarray(inputs["hidden_states"], np.float32)[0]      # [S, C]
    enc = np.asarray(inputs["encoder_hidden_states"], np.float32)[0]
    emb = np.asarray(inputs["embedded_timestep"], np.float32)[0]  # [C]
    temb = np.asarray(inputs["temb"], np.float32)                # [1, 3C]
    cosT = np.ascontiguousarray(np.asarray(inputs["rope_cos"],
                                           np.float32).T)        # [D, S]
    sinT = np.ascontiguousarray(np.asarray(inputs["rope_sin"],
                                           np.float32).T)

    for k in ("attn1_qn", "attn1_kn", "attn2_qn", "attn2_kn"):
        assert np.allclose(np.asarray(inputs[k]), 1.0), f"{k} != ones"

    rot = np.zeros((D, D), np.float32)  # rot_out = rot_m.T @ q
    for d in range(D // 2):
        rot[64 + d, d] = -1.0
        rot[d, 64 + d] = 1.0

    aw2 = [np.asarray(inputs[f"a{i}_w2"], np.float32) for i in (1, 2, 3)]
    common = {
        "wq1": _bf(inputs["attn1_wq"]), "wk1": _bf(inputs["attn1_wk"]),
        "wv1": _bf(inputs["attn1_wv"]), "wo1": _bf(inputs["attn1_wo"]),
        "wq2": _bf(inputs["attn2_wq"]), "wo2": _bf(inputs["attn2_wo"]),
        "wk2": _bf(inputs["attn2_wk"]), "wv2": _bf(inputs["attn2_wv"]),
        "w1f": _bf(inputs["ff_w1"]), "w2f": _bf(inputs["ff_w2"]),
        "encT": _bf(enc.T),
        "aw1": _bf(np.stack([inputs["a1_w1"], inputs["a2_w1"],
                             inputs["a3_w1"]])),
        "emb_t": np.ascontiguousarray(emb.reshape(CT, 128).T),
        "rotm": round_fp32r(rot),
        "eye": round_fp32r(np.eye(128, dtype=np.float32)),
        "onec": np.ones((128, 1), np.float32),
        "onecb": np.ones((128, 1), np.float32).astype(ml_dtypes.bfloat16),
    }
    in_maps = []
    for r in range(NCORES):
        tok = slice(SL * r, SL * (r + 1))
        hd = slice(256 * r, 256 * (r + 1))
        m = dict(common)
        m["h_s"] = np.ascontiguousarray(h[tok, :])
        m["cosT"] = np.ascontiguousarray(cosT[:, tok])
        m["sinT"] = np.ascontiguousarray(sinT[:, tok])
        m["aw2l"] = _bf(np.stack([
            np.concatenate([a[:, 2048 * t + 256 * r:2048 * t + 256 * (r + 1)]
                            for t in range(3)], axis=1) for a in aw2]))
        m["temb_l"] = np.ascontiguousarray(np.stack(
            [temb[0, 2048 * t + 256 * r:2048 * t + 256 * (r + 1)]
             for t in range(3)]))
        in_maps.append(m)

    nc = _get_nc()
    res = run_bass_kernel_spmd(nc, in_maps, core_ids=list(range(NCORES)))
    out = np.concatenate([res.results[i]["h_out"] for i in range(NCORES)],
                         axis=0)
    return out.reshape(1, S, C).astype(np.float32)


if __name__ == "__main__":
    _get_nc()
    print("build + compile OK")


# revision 3
# speedup vs baseline: 1029.4982x; 1.0248x over previous
"""CosmosTransformerBlock on 8 Trainium2 NeuronCores — sequence-parallel.

Strategy v2 (sequence-parallel, replicated weights, bf16 matmuls):
- Each core owns SL=256 tokens of the residual stream; LayerNorm/AdaLN
  modulation, QKV, attention queries, wo, and the whole FF run locally on
  those tokens with full (replicated) weight matrices — identical FLOPs to
  tensor-parallel, but attn2 and FF need NO collectives at all.
- Self-attention needs full-sequence K/V: one fused AllGather of the local
  [K^T; V] buffer (bf16). Cross-attention K/V is computed head-sharded from
  the encoder (2 heads/core) and AllGather'd early (overlaps sublayer 0).
- AdaLN lora-up matmul is sharded 768 cols/core; the tiny e vectors are
  AllGather'd at kernel start.
- All matmuls run in bf16 (1 cyc/row on PE) with fp32 PSUM accumulation;
  residual h, LN stats, softmax normalization stay fp32.
"""

import sys

import numpy as np

try:
    import concourse.bass as bass
except ImportError:  # pragma: no cover
    sys.path.insert(0, "/opt/trn_rl_repo")
    import concourse.bass as bass

import ml_dtypes
import concourse.mybir as mybir
import concourse.tile as tile
from concourse import bacc
from concourse.bass_utils import run_bass_kernel_spmd

F32 = mybir.dt.float32
F32R = mybir.dt.float32r
BF16 = mybir.dt.bfloat16
AF = mybir.ActivationFunctionType
ALU = mybir.AluOpType

NCORES = 8
S, C, D, H = 2048, 2048, 128, 16
SL = S // NCORES      # 256 tokens per core
CT = C // 128         # 16 feature tiles
SC, CROSS, LORA, FF = 512, 1024, 256, 8192
FT = FF // 128        # 64 ff tiles
EPS = 1e-6
ISQD = float(D) ** -0.5

KV1K = H * D * SL     # 524288: K^T elems in kv1_in
KV1SZ = 2 * KV1K
KV2K = 2 * D * SC     # 131072: local-2-head K2^T elems in kv2_in
KV2SZ = 2 * KV2K
RG = [list(range(NCORES))]


def round_fp32r(a: np.ndarray) -> np.ndarray:
    b = np.ascontiguousarray(a, dtype=np.float32).view(np.uint32)
    lsb = (b >> np.uint32(12)) & np.uint32(1)
    r = (b + np.uint32(0x7FF) + lsb) & np.uint32(0xFFFFF000)
    return r.view(np.float32)


def _build(nc: bacc.Bacc):
    dram = lambda n, s, d: nc.dram_tensor(n, s, d, kind="ExternalInput").ap()

    h_in = dram("h_s", [SL, C], F32)
    wq1 = dram("wq1", [C, C], BF16)
    wk1 = dram("wk1", [C, C], BF16)
    wv1 = dram("wv1", [C, C], BF16)
    wo1 = dram("wo1", [C, C], BF16)
    wq2 = dram("wq2", [C, C], BF16)
    wk2 = dram("wk2", [CROSS, C], BF16)
    wv2 = dram("wv2", [CROSS, C], BF16)
    wo2 = dram("wo2", [C, C], BF16)
    w1f = dram("w1f", [C, FF], BF16)
    w2f = dram("w2f", [FF, C], BF16)
    encT = dram("encT", [CROSS, SC], BF16)
    aw1 = dram("aw1", [3, C, LORA], BF16)
    aw2l = dram("aw2l", [3, LORA, 768], BF16)
    temb_l = dram("temb_l", [3, 256], F32)
    emb_t = dram("emb_t", [128, CT], F32)
    cosT = dram("cosT", [D, SL], F32)
    sinT = dram("sinT", [D, SL], F32)
    rotm = dram("rotm", [D, D], F32R)
    eye = dram("eye", [128, 128], F32R)
    onec = dram("onec", [128, 1], F32R)
    onecb = dram("onecb", [128, 1], BF16)

    h_out = nc.dram_tensor("h_out", [SL, C], F32, kind="ExternalOutput").ap()

    e_in = nc.dram_tensor("e_in", [9, 256], F32).ap()
    e_out = nc.dram_tensor("e_out", [NCORES * 9, 256], F32,
                           addr_space="Shared").ap()
    kv2d = nc.dram_tensor("kv2d", [2 * 128 * H * SC], BF16).ap()
    kv1_in = nc.dram_tensor("kv1_in", [KV1SZ], BF16).ap()
    kv1_out = nc.dram_tensor("kv1_out", [NCORES * KV1SZ], BF16,
                             addr_space="Shared").ap()

    with tile.TileContext(nc) as tc:
        ctx_pool = tc.tile_pool(name="persist", bufs=1)
        persist = ctx_pool.__enter__()

        h_sb = persist.tile([128, 2, C], F32, tag="h")
        nc.sync.dma_start(out=h_sb, in_=h_in.rearrange("(u p) c -> p u c",
                                                       p=128))
        eye_sb = persist.tile([128, 128], F32R, tag="eye")
        nc.sync.dma_start(out=eye_sb, in_=eye)
        onec_sb = persist.tile([128, 1], F32R, tag="onec")
        nc.sync.dma_start(out=onec_sb, in_=onec)
        onecb_sb = persist.tile([128, 1], BF16, tag="onecb")
        nc.sync.dma_start(out=onecb_sb, in_=onecb)
        rotm_sb = persist.tile([D, D], F32R, tag="rotm")
        nc.sync.dma_start(out=rotm_sb, in_=rotm)
        emb_sb = persist.tile([128, CT], F32, tag="emb")
        nc.sync.dma_start(out=emb_sb, in_=emb_t)
        eps_sb = persist.tile([128, 1], F32, tag="eps")
        nc.vector.memset(eps_sb, EPS)
        # cos/sin tiled x2 for 2-head batched rope epilogues
        cos2_sb = persist.tile([D, 2, SL], F32, tag="cos2")
        nc.sync.dma_start(out=cos2_sb, in_=bass.AP(
            tensor=cosT.tensor, offset=0, ap=[[SL, 128], [0, 2], [1, SL]]))
        sin2_sb = persist.tile([D, 2, SL], F32, tag="sin2")
        nc.sync.dma_start(out=sin2_sb, in_=bass.AP(
            tensor=sinT.tensor, offset=0, ap=[[SL, 128], [0, 2], [1, SL]]))
        gateB = persist.tile([128, C], F32, tag="gateB")
        q_sb = persist.tile([128, H, SL], BF16, tag="q")
        xnT_sb = persist.tile([128, CT, SL], BF16, tag="xnT")
        a_sb = persist.tile([128, H, SL], BF16, tag="a")

        # ---------------- AdaLN (sharded 768 cols/core) + e AllGather -------
        # All three sublayers' lora-down products y_L are computed first and
        # round-trip DRAM once; then the lora-up slices + temb -> e AllGather.
        with tc.tile_pool(name="adaln", bufs=3) as apool, \
             tc.tile_pool(name="adaln_ps", bufs=2, space="PSUM") as apsum:
            silu_sb = apool.tile([128, CT], BF16, tag="silu", bufs=1)
            nc.scalar.activation(out=silu_sb, in_=emb_sb, func=AF.Silu)
            temb_sb = apool.tile([1, 3, 256], F32, tag="tembl", bufs=1)
            nc.sync.dma_start(out=temb_sb, in_=bass.AP(
                tensor=temb_l.tensor, offset=0,
                ap=[[768, 1], [256, 3], [1, 256]]))
            e_loc = apool.tile([1, 9, 256], F32, tag="eloc", bufs=1)
            yall = apool.tile([1, 3, LORA], BF16, tag="yall", bufs=1)
            for L in range(3):
                aw1_sb = apool.tile([128, CT, LORA], BF16, tag="aw1")
                nc.sync.dma_start(
                    out=aw1_sb, in_=aw1[L].rearrange("(t p) m -> p t m",
                                                     p=128))
                y_ps = apsum.tile([1, LORA], F32, tag="yps")
                for ct in range(CT):
                    nc.tensor.matmul(y_ps, lhsT=silu_sb[:, ct:ct + 1],
                                     rhs=aw1_sb[:, ct, :],
                                     start=(ct == 0), stop=(ct == CT - 1))
                nc.scalar.copy(out=yall[:, L, :], in_=y_ps)
            # redistribute y to partitions via tiny PE transposes (no DRAM
            # round trip): y2[:, L, t2] = yall[0, L, 128*t2 : 128*(t2+1)]^T
            y2 = apool.tile([128, 3, 2], BF16, tag="y2", bufs=1)
            for L in range(3):
                for t2 in range(2):
                    y2p = apsum.tile([128, 1], F32, tag="y2p")
                    nc.tensor.matmul(
                        y2p, lhsT=yall[:, L, 128 * t2:128 * (t2 + 1)],
                        rhs=onecb_sb[0:1, :], start=True, stop=True)
                    nc.vector.tensor_copy(out=y2[:, L, t2:t2 + 1], in_=y2p)
            for L in range(3):
                aw2_sb = apool.tile([128, 2, 3, 256], BF16, tag="aw2")
                nc.sync.dma_start(
                    out=aw2_sb,
                    in_=aw2l[L].rearrange("(t p) (g n) -> p t g n",
                                          p=128, n=256))
                for g3 in range(3):
                    e_ps = apsum.tile([1, 256], F32, tag="eps2")
                    nc.tensor.matmul(e_ps, lhsT=y2[:, L, 0:1],
                                     rhs=aw2_sb[:, 0, g3, :],
                                     start=True, stop=False)
                    nc.tensor.matmul(e_ps, lhsT=y2[:, L, 1:2],
                                     rhs=aw2_sb[:, 1, g3, :],
                                     start=False, stop=True)
                    nc.vector.tensor_add(out=e_loc[:, 3 * L + g3, :],
                                         in0=e_ps,
                                         in1=temb_sb[:, g3, :])
            nc.sync.dma_start(out=bass.AP(
                tensor=e_in.tensor, offset=0,
                ap=[[2304, 1], [256, 9], [1, 256]]), in_=e_loc)
        nc.gpsimd.collective_compute(
            "AllGather", ALU.bypass, replica_groups=RG,
            ins=[e_in], outs=[e_out])

        def _ebcast(L, t):
            # [128, 8, 256] broadcast view of e_out for (sublayer L, type t)
            return bass.AP(tensor=e_out.tensor, offset=(3 * L + t) * 256,
                           ap=[[0, 128], [9 * 256, 8], [1, 256]])

        # ---------------- modulate: LN + transpose + feature-major mod ------
        # LN and the transposes do NOT wait for the e AllGather; scale/shift
        # are applied per-partition (feature) on the transposed tiles.
        def modulate(L, mpool, mpsum):
            shT = mpool.tile([128, 2, 8], F32, tag="shT", bufs=1)
            opT = mpool.tile([128, 2, 8], F32, tag="opT", bufs=1)
            for par in range(2):
                nc.sync.dma_start(out=shT[:, par, :], in_=bass.AP(
                    tensor=e_out.tensor, offset=3 * L * 256 + par * 128,
                    ap=[[1, 128], [2304, 8]]))
                nc.sync.dma_start(out=opT[:, par, :], in_=bass.AP(
                    tensor=e_out.tensor, offset=(3 * L + 1) * 256 + par * 128,
                    ap=[[1, 128], [2304, 8]]))
            nc.vector.tensor_scalar_add(out=opT, in0=opT, scalar1=1.0)
            nc.sync.dma_start(out=gateB, in_=_ebcast(L, 2))
            for u in range(2):
                x = h_sb[:, u, :]
                stats = mpool.tile([128, 4, 6], F32, tag="stats")
                xv = x.rearrange("p (g n) -> p g n", n=512)
                for g in range(4):
                    nc.vector.bn_stats(out=stats[:, g, :], in_=xv[:, g, :])
                mv = mpool.tile([128, 2], F32, tag="mv")
                nc.vector.bn_aggr(out=mv, in_=stats)
                sd = mpool.tile([128, 1], F32, tag="sd")
                nc.scalar.activation(out=sd, in_=mv[:, 1:2], func=AF.Sqrt,
                                     bias=eps_sb)
                rstd = mpool.tile([128, 1], F32, tag="rstd")
                nc.vector.reciprocal(out=rstd, in_=sd)
                t1 = mpool.tile([128, C], F32R, tag="mod_t1",
                                name=f"t1_{u}")
                nc.vector.tensor_scalar(out=t1, in0=x, scalar1=mv[:, 0:1],
                                        scalar2=rstd, op0=ALU.subtract,
                                        op1=ALU.mult)
                for ct in range(CT):
                    tp = mpsum.tile([128, 128], F32R, tag="tpsum")
                    nc.tensor.transpose(
                        tp, t1[:, 128 * ct:128 * (ct + 1)], eye_sb)
                    nc.vector.tensor_scalar(
                        out=xnT_sb[:, ct, 128 * u:128 * (u + 1)], in0=tp,
                        scalar1=opT[:, ct % 2, ct // 2:ct // 2 + 1],
                        scalar2=shT[:, ct % 2, ct // 2:ct // 2 + 1],
                        op0=ALU.mult, op1=ALU.add)

        # batched rms(+rope) epilogue: src sbuf f32r [128, 2, 256] (2 heads)
        def rms_ep(src, dst, rope, pool, psum):
            sq = pool.tile([128, 2 * SL], F32R, tag="ep_sq")
            nc.scalar.activation(out=sq, in_=src, func=AF.Square)
            ss = psum.tile([1, 2 * SL], F32, tag="ep_ss")
            nc.tensor.matmul(ss, lhsT=onec_sb, rhs=sq, start=True, stop=True)
            sd = pool.tile([1, 2 * SL], F32, tag="ep_sd")
            nc.scalar.activation(out=sd, in_=ss, func=AF.Sqrt, scale=1.0 / D,
                                 bias=eps_sb[0:1, :])
            rc = pool.tile([1, 2 * SL], F32, tag="ep_rc")
            nc.vector.reciprocal(out=rc, in_=sd)
            rb = pool.tile([128, 2 * SL], F32, tag="ep_rb")
            nc.gpsimd.partition_broadcast(out_ap=rb, in_ap=rc)
            if rope:
                rot = psum.tile([128, 2 * SL], F32, tag="ep_rot")
                nc.tensor.matmul(rot, lhsT=rotm_sb, rhs=src,
                                 start=True, stop=True)
                t1 = pool.tile([128, 2 * SL], F32, tag="ep_t1")
                nc.vector.tensor_mul(out=t1, in0=src, in1=cos2_sb)
                t2 = pool.tile([128, 2 * SL], F32, tag="ep_t2")
                nc.vector.tensor_mul(out=t2, in0=rot, in1=sin2_sb)
                nc.vector.tensor_add(out=t1, in0=t1, in1=t2)
                nc.vector.tensor_mul(out=dst, in0=t1, in1=rb)
            else:
                nc.vector.tensor_mul(out=dst, in0=src, in1=rb)

        # wo + gated residual (shared by attn1/attn2); first chunks may
        # be prefetched into `pre` while the preceding phase runs
        def wo_prefetch(wX, pool, n=2):
            wv_ = wX.rearrange("(t p) (cc m) -> p cc t m", p=128, m=512)
            pre = []
            for cc in range(n):
                wo_sb = pool.tile([128, CT, 512], BF16, tag="wos")
                nc.sync.dma_start(out=wo_sb, in_=wv_[:, cc])
                pre.append(wo_sb)
            return wv_, pre

        def wo_residual(wv_, pool, pre):
            with tc.tile_pool(name="wops", bufs=2, space="PSUM") as psum:
                for cc in range(4):
                    if cc < len(pre):
                        wo_sb = pre[cc]
                    else:
                        wo_sb = pool.tile([128, CT, 512], BF16, tag="wos")
                        nc.sync.dma_start(out=wo_sb, in_=wv_[:, cc])
                    for u in range(2):
                        ops = psum.tile([128, 512], F32, tag="wops")
                        for hh in range(H):
                            nc.tensor.matmul(
                                ops,
                                lhsT=a_sb[:, hh, 128 * u:128 * (u + 1)],
                                rhs=wo_sb[:, hh, :],
                                start=(hh == 0), stop=(hh == H - 1))
                        gt = pool.tile([128, 512], F32, tag="wogt")
                        nc.vector.tensor_mul(
                            out=gt, in0=ops,
                            in1=gateB[:, 512 * cc:512 * (cc + 1)])
                        nc.vector.tensor_add(
                            out=h_sb[:, u, 512 * cc:512 * (cc + 1)],
                            in0=h_sb[:, u, 512 * cc:512 * (cc + 1)], in1=gt)

        # ======================= sublayer 0: self attention ================
        with tc.tile_pool(name="mod0", bufs=2) as mpool, \
             tc.tile_pool(name="mod0ps", bufs=2, space="PSUM") as mpsum:
            modulate(0, mpool, mpsum)

        # K/V in 4 head-groups of 4, written to k1_in/v1_in, then AllGathers
        wq1v = wq1.rearrange("(t p) (g m) -> p g t m", p=128, m=512)
        wk1v = wk1.rearrange("(t p) (g m) -> p g t m", p=128, m=512)
        wv1v = wv1.rearrange("(t p) (g m) -> p g t m", p=128, m=512)
        with tc.tile_pool(name="kv1w", bufs=2) as wpool, \
             tc.tile_pool(name="kv1e", bufs=1) as epool, \
             tc.tile_pool(name="kv1ps", bufs=1, space="PSUM") as qpsum, \
             tc.tile_pool(name="kv1ps2", bufs=1, space="PSUM") as qpsum2:
            # K pass first: the long rms+rope epilogue chains then overlap
            # the V matmul pass, so the AllGather is gated only by V's short
            # eviction chain.
            for g in range(4):
                wk_sb = wpool.tile([128, CT, 512], BF16, tag="wk")
                nc.sync.dma_start(out=wk_sb, in_=wk1v[:, g])
                pk = qpsum.tile([128, 4, SL], F32, tag="pk", bufs=2)
                for ct in range(CT):
                    st, sp = (ct == 0), (ct == CT - 1)
                    for j in range(4):
                        nc.tensor.matmul(
                            pk[:, j, :],
                            lhsT=wk_sb[:, ct, 128 * j:128 * (j + 1)],
                            rhs=xnT_sb[:, ct, :], start=st, stop=sp)
                kf = epool.tile([128, 4, SL], F32R, tag="kf")
                nc.vector.tensor_copy(out=kf, in_=pk)
                for jj in range(2):
                    ke = epool.tile([128, 2, SL], BF16, tag="ke")
                    rms_ep(kf[:, 2 * jj:2 * jj + 2, :], ke, True,
                           epool, qpsum2)
                    nc.sync.dma_start(
                        out=bass.AP(
                            tensor=kv1_in.tensor,
                            offset=(4 * g + 2 * jj) * D * SL,
                            ap=[[SL, 128], [D * SL, 2], [1, SL]]),
                        in_=ke)
            for g in range(4):
                wv_sb = wpool.tile([128, CT, 512], BF16, tag="wv")
                nc.sync.dma_start(out=wv_sb, in_=wv1v[:, g])
                pv = qpsum.tile([128, 2, 512], F32, tag="pv")
                for ct in range(CT):
                    st, sp = (ct == 0), (ct == CT - 1)
                    for u in range(2):
                        nc.tensor.matmul(
                            pv[:, u, :],
                            lhsT=xnT_sb[:, ct, 128 * u:128 * (u + 1)],
                            rhs=wv_sb[:, ct, :], start=st, stop=sp)
                ve = epool.tile([128, 2, 512], BF16, tag="ve")
                nc.vector.tensor_copy(out=ve, in_=pv)
                nc.sync.dma_start(
                    out=bass.AP(tensor=kv1_in.tensor,
                                offset=KV1K + 512 * g,
                                ap=[[C, 128], [128 * C, 2], [1, 512]]),
                    in_=ve)
        nc.gpsimd.collective_compute(
            "AllGather", ALU.bypass, replica_groups=RG,
            ins=[kv1_in], outs=[kv1_out])

        # ---- during the AllGather window: Q (rms+rope) and enc K2/V2 ------
        with tc.tile_pool(name="q1w", bufs=2) as wpool, \
             tc.tile_pool(name="q1e", bufs=1) as epool, \
             tc.tile_pool(name="q1ps", bufs=1, space="PSUM") as qpsum, \
             tc.tile_pool(name="q1ps2", bufs=1, space="PSUM") as qpsum2:
            for g in range(4):
                wq_sb = wpool.tile([128, CT, 512], BF16, tag="wq")
                nc.scalar.dma_start(out=wq_sb, in_=wq1v[:, g])
                pq = qpsum.tile([128, 4, SL], F32, tag="pq")
                for ct in range(CT):
                    st, sp = (ct == 0), (ct == CT - 1)
                    for j in range(4):
                        nc.tensor.matmul(
                            pq[:, j, :],
                            lhsT=wq_sb[:, ct, 128 * j:128 * (j + 1)],
                            rhs=xnT_sb[:, ct, :], start=st, stop=sp)
                qf = epool.tile([128, 4, SL], F32R, tag="qf")
                nc.vector.tensor_copy(out=qf, in_=pq)
                for jj in range(2):
                    h0 = 4 * g + 2 * jj
                    rms_ep(qf[:, 2 * jj:2 * jj + 2, :],
                           q_sb[:, h0:h0 + 2, :], True, epool, qpsum2)

        # replicated cross-attn K2^T/V2 from the encoder (all 16 heads);
        # results are spilled to DRAM and reloaded at sdpa2 to keep SBUF free
        wk2v = wk2.rearrange("(t p) (g m) -> p g t m", p=128, m=256)
        wv2v = wv2.rearrange("(t p) (g m) -> p g t m", p=128, m=256)
        with tc.tile_pool(name="enckv", bufs=2) as kpool, \
             tc.tile_pool(name="enckvps", bufs=1, space="PSUM") as kpsum, \
             tc.tile_pool(name="enckvps2", bufs=1, space="PSUM") as kpsum2:
            k2_sb = kpool.tile([128, H, SC], BF16, tag="k2", bufs=1)
            v2_sb = kpool.tile([128, 4, H, D], BF16, tag="v2", bufs=1)
            enc_sb = kpool.tile([128, 8, SC], BF16, tag="enc", bufs=1)
            nc.scalar.dma_start(out=enc_sb,
                              in_=encT.rearrange("(t p) s -> p t s", p=128))
            for g in range(8):  # pairs of heads
                wk2_sb = kpool.tile([128, 8, 256], BF16, tag="wk2")
                nc.scalar.dma_start(out=wk2_sb, in_=wk2v[:, g])
                wv2_sb = kpool.tile([128, 8, 256], BF16, tag="wv2")
                nc.scalar.dma_start(out=wv2_sb, in_=wv2v[:, g])
                pk2 = kpsum.tile([128, 2, SC], F32, tag="pk2")
                pv2 = kpsum.tile([128, 4, 256], F32, tag="pv2")
                for t in range(8):
                    st, sp = (t == 0), (t == 7)
                    for j in range(2):
                        nc.tensor.matmul(
                            pk2[:, j, :],
                            lhsT=wk2_sb[:, t, 128 * j:128 * (j + 1)],
                            rhs=enc_sb[:, t, :], start=st, stop=sp)
                    for tt in range(4):
                        nc.tensor.matmul(
                            pv2[:, tt, :],
                            lhsT=enc_sb[:, t, 128 * tt:128 * (tt + 1)],
                            rhs=wv2_sb[:, t, :], start=st, stop=sp)
                kf2 = kpool.tile([128, 2, SC], F32R, tag="kf2")
                nc.vector.tensor_copy(out=kf2, in_=pk2)
                # batched rms over head dim (partition), no rope
                sq = kpool.tile([128, 2, SC], F32R, tag="k2sq")
                nc.scalar.activation(out=sq, in_=kf2, func=AF.Square)
                ss = kpsum2.tile([1, 2, SC], F32, tag="k2ss")
                for j in range(2):
                    nc.tensor.matmul(ss[:, j, :], lhsT=onec_sb,
                                     rhs=sq[:, j, :], start=True, stop=True)
                sd = kpool.tile([1, 2 * SC], F32, tag="k2sd")
                nc.scalar.activation(out=sd, in_=ss, func=AF.Sqrt,
                                     scale=1.0 / D, bias=eps_sb[0:1, :])
                rc = kpool.tile([1, 2 * SC], F32, tag="k2rc")
                nc.vector.reciprocal(out=rc, in_=sd)
                rb = kpool.tile([128, 2 * SC], F32, tag="k2rb")
                nc.gpsimd.partition_broadcast(out_ap=rb, in_ap=rc)
                nc.vector.tensor_mul(out=k2_sb[:, 2 * g:2 * g + 2, :],
                                     in0=kf2, in1=rb)
                nc.vector.tensor_copy(out=v2_sb[:, :, 2 * g:2 * g + 2, :],
                                      in_=pv2)
            nc.sync.dma_start(out=bass.AP(
                tensor=kv2d.tensor, offset=0,
                ap=[[H * SC, 128], [1, H * SC]]), in_=k2_sb)
            nc.sync.dma_start(out=bass.AP(
                tensor=kv2d.tensor, offset=128 * H * SC,
                ap=[[H * SC, 128], [1, H * SC]]), in_=v2_sb)

        # sdpa over 2 halves of 8 heads; 4-head interleaved softmax rounds.
        # PE emission is software-pipelined: scores(kt) are issued before
        # den/av(kt-1) so PE never stalls on the Exp of the current round.
        def sdpa1(spool, spsum, spsum2):
            for quarter in range(4):
                hb0 = 4 * quarter
                k_sb = spool.tile([128, 4, 8, SL], BF16, tag="ksb", bufs=2)
                v_sb = spool.tile([128, 16, 512], BF16, tag="vsb", bufs=2)
                for r in range(NCORES):
                    nc.sync.dma_start(out=k_sb[:, :, r, :], in_=bass.AP(
                        tensor=kv1_out.tensor,
                        offset=hb0 * D * SL + r * KV1SZ,
                        ap=[[SL, 128], [D * SL, 4], [1, SL]]))
                    nc.sync.dma_start(out=v_sb[:, 2 * r:2 * r + 2, :],
                                      in_=bass.AP(
                        tensor=kv1_out.tensor,
                        offset=KV1K + hb0 * D + r * KV1SZ,
                        ap=[[C, 128], [128 * C, 2], [1, 512]]))
                if True:
                    hb = 0
                    av = spsum.tile([128, 4, SL], F32, tag="av")
                    den = spsum.tile([1, 2, 2 * SL], F32, tag="den")

                    def sc_round(kt):
                        sps = spsum2.tile([128, 4, SL], F32, tag="sps",
                                          name=f"sps{kt}")
                        for j in range(4):
                            nc.tensor.matmul(
                                sps[:, j, :],
                                lhsT=k_sb[:, j, kt // 2,
                                          128 * (kt % 2):128 * (kt % 2) + 128],
                                rhs=q_sb[:, hb0 + j, :],
                                start=True, stop=True)
                        pt = spool.tile([128, 4, SL], BF16, tag="pt",
                                        name=f"pt{kt}")
                        nc.scalar.activation(out=pt, in_=sps, func=AF.Exp,
                                             scale=ISQD)
                        return pt

                    def av_round(kt, pt):
                        st, sp = (kt == 0), (kt == 15)
                        for dh in range(2):
                            nc.tensor.matmul(
                                den[:, dh, :], lhsT=onecb_sb,
                                rhs=pt[:, 2 * dh:2 * dh + 2, :],
                                start=st, stop=sp)
                        for j in range(4):
                            nc.tensor.matmul(
                                av[:, j, :],
                                lhsT=v_sb[:, kt, 128 * j:128 * (j + 1)],
                                rhs=pt[:, j, :], start=st, stop=sp)

                    prev = sc_round(0)
                    for kt in range(1, 16):
                        cur = sc_round(kt)
                        av_round(kt - 1, prev)
                        prev = cur
                    av_round(15, prev)
                    rc = spool.tile([1, 4 * SL], F32, tag="sd_rc")
                    nc.vector.reciprocal(out=rc, in_=den)
                    rb = spool.tile([128, 4 * SL], F32, tag="sd_rb")
                    nc.gpsimd.partition_broadcast(out_ap=rb, in_ap=rc)
                    nc.vector.tensor_mul(
                        out=a_sb[:, hb0:hb0 + 4, :],
                        in0=av, in1=rb)

        wo1_ctx = tc.tile_pool(name="wo1", bufs=2)
        wo1_pool = wo1_ctx.__enter__()
        wo1v, wo1_pre = wo_prefetch(wo1, wo1_pool)

        with tc.tile_pool(name="sdpa1", bufs=3) as spool, \
             tc.tile_pool(name="sdpa1ps", bufs=1, space="PSUM") as spsum, \
             tc.tile_pool(name="sdpa1ps2", bufs=2, space="PSUM") as spsum2:
            sdpa1(spool, spsum, spsum2)

        wo_residual(wo1v, wo1_pool, wo1_pre)
        wo1_ctx.__exit__(None, None, None)

        # ======================= sublayer 1: cross attention ===============
        with tc.tile_pool(name="mod1", bufs=2) as mpool, \
             tc.tile_pool(name="mod1ps", bufs=2, space="PSUM") as mpsum:
            modulate(1, mpool, mpsum)

        wq2v = wq2.rearrange("(t p) (g m) -> p g t m", p=128, m=512)
        with tc.tile_pool(name="q2w", bufs=2) as wpool, \
             tc.tile_pool(name="q2e", bufs=1) as epool, \
             tc.tile_pool(name="q2ps", bufs=1, space="PSUM") as qpsum, \
             tc.tile_pool(name="q2ps2", bufs=1, space="PSUM") as qpsum2:
            for g in range(4):
                wq_sb = wpool.tile([128, CT, 512], BF16, tag="wq2")
                nc.scalar.dma_start(out=wq_sb, in_=wq2v[:, g])
                pq = qpsum.tile([128, 4, SL], F32, tag="pq2")
                for ct in range(CT):
                    st, sp = (ct == 0), (ct == CT - 1)
                    for j in range(4):
                        nc.tensor.matmul(
                            pq[:, j, :],
                            lhsT=wq_sb[:, ct, 128 * j:128 * (j + 1)],
                            rhs=xnT_sb[:, ct, :], start=st, stop=sp)
                qf = epool.tile([128, 4, SL], F32R, tag="qf2")
                nc.vector.tensor_copy(out=qf, in_=pq)
                for jj in range(2):
                    h0 = 4 * g + 2 * jj
                    rms_ep(qf[:, 2 * jj:2 * jj + 2, :],
                           q_sb[:, h0:h0 + 2, :], False, epool, qpsum2)

        def sdpa2(spool, spsum, spsum2):
            k2_sb = spool.tile([128, H, SC], BF16, tag="k2r", bufs=1)
            nc.sync.dma_start(out=k2_sb, in_=bass.AP(
                tensor=kv2d.tensor, offset=0,
                ap=[[H * SC, 128], [1, H * SC]]))
            v2_sb = spool.tile([128, 4, H, D], BF16, tag="v2r", bufs=1)
            nc.sync.dma_start(out=v2_sb, in_=bass.AP(
                tensor=kv2d.tensor, offset=128 * H * SC,
                ap=[[H * SC, 128], [1, H * SC]]))
            for sg in range(4):
                hb = 4 * sg
                av = spsum.tile([128, 4, SL], F32, tag="av2")
                den = spsum.tile([1, 2, 2 * SL], F32, tag="den2")

                def sc_round(kt):
                    sps = spsum2.tile([128, 4, SL], F32, tag="sps2",
                                      name=f"sps2_{kt}")
                    for j in range(4):
                        nc.tensor.matmul(
                            sps[:, j, :],
                            lhsT=k2_sb[:, hb + j, 128 * kt:128 * (kt + 1)],
                            rhs=q_sb[:, hb + j, :], start=True, stop=True)
                    pt = spool.tile([128, 4, SL], BF16, tag="pt2",
                                    name=f"pt2_{kt}")
                    nc.scalar.activation(out=pt, in_=sps, func=AF.Exp,
                                         scale=ISQD)
                    return pt

                def av_round(kt, pt):
                    st, sp = (kt == 0), (kt == 3)
                    for dh in range(2):
                        nc.tensor.matmul(
                            den[:, dh, :], lhsT=onecb_sb,
                            rhs=pt[:, 2 * dh:2 * dh + 2, :],
                            start=st, stop=sp)
                    for j in range(4):
                        nc.tensor.matmul(
                            av[:, j, :],
                            lhsT=v2_sb[:, kt, hb + j, :],
                            rhs=pt[:, j, :], start=st, stop=sp)

                prev = sc_round(0)
                for kt in range(1, 4):
                    cur = sc_round(kt)
                    av_round(kt - 1, prev)
                    prev = cur
                av_round(3, prev)
                rc = spool.tile([1, 4 * SL], F32, tag="sd2_rc")
                nc.vector.reciprocal(out=rc, in_=den)
                rb = spool.tile([128, 4 * SL], F32, tag="sd2_rb")
                nc.gpsimd.partition_broadcast(out_ap=rb, in_ap=rc)
                nc.vector.tensor_mul(out=a_sb[:, hb:hb + 4, :],
                                     in0=av, in1=rb)

        w1v = w1f.rearrange("(t p) (fg m) -> p fg t m", p=128, m=512)
        ff1_ctx = tc.tile_pool(name="ff1", bufs=2)
        ff1_pool = ff1_ctx.__enter__()
        wo2_ctx = tc.tile_pool(name="wo2", bufs=2)
        wo2_pool = wo2_ctx.__enter__()
        wo2v, wo2_pre = wo_prefetch(wo2, wo2_pool)
        w1_pre = []
        for fg in range(2):
            w1_sb = ff1_pool.tile([128, CT, 512], BF16, tag="w1")
            nc.sync.dma_start(out=w1_sb, in_=w1v[:, fg])
            w1_pre.append(w1_sb)

        with tc.tile_pool(name="sdpa2", bufs=3) as spool, \
             tc.tile_pool(name="sdpa2ps", bufs=1, space="PSUM") as spsum, \
             tc.tile_pool(name="sdpa2ps2", bufs=2, space="PSUM") as spsum2:
            sdpa2(spool, spsum, spsum2)

        wo_residual(wo2v, wo2_pool, wo2_pre)
        wo2_ctx.__exit__(None, None, None)

        # ======================= sublayer 2: feed forward ==================
        with tc.tile_pool(name="mod2", bufs=2) as mpool, \
             tc.tile_pool(name="mod2ps", bufs=2, space="PSUM") as mpsum:
            modulate(2, mpool, mpsum)

        w2v = w2f.rearrange("(f p) (cc m) -> p cc f m", p=128, m=256)
        with tc.tile_pool(name="ffg", bufs=1) as gpool:
            g_sb = gpool.tile([128, FT, SL], BF16, tag="g")
            with tc.tile_pool(name="ff1ps", bufs=4, space="PSUM") as fpsum:
                for fg in range(16):
                    if fg < len(w1_pre):
                        w1_sb = w1_pre[fg]
                    else:
                        w1_sb = ff1_pool.tile([128, CT, 512], BF16, tag="w1")
                        nc.sync.dma_start(out=w1_sb, in_=w1v[:, fg])
                    for ft in range(4):
                        gps = fpsum.tile([128, SL], F32, tag="gps")
                        for ct in range(CT):
                            nc.tensor.matmul(
                                gps,
                                lhsT=w1_sb[:, ct, 128 * ft:128 * (ft + 1)],
                                rhs=xnT_sb[:, ct, :],
                                start=(ct == 0), stop=(ct == CT - 1))
                        nc.scalar.activation(
                            out=g_sb[:, 4 * fg + ft, :], in_=gps,
                            func=AF.Gelu)
            with tc.tile_pool(name="ff2", bufs=2) as wpool, \
                 tc.tile_pool(name="ff2ps", bufs=2, space="PSUM") as fpsum:
                for cc in range(8):
                    w2_sb = wpool.tile([128, FT, 256], BF16, tag="w2")
                    nc.sync.dma_start(out=w2_sb, in_=w2v[:, cc])
                    for u in range(2):
                        yps = fpsum.tile([128, 256], F32, tag="yps")
                        for f in range(FT):
                            nc.tensor.matmul(
                                yps,
                                lhsT=g_sb[:, f, 128 * u:128 * (u + 1)],
                                rhs=w2_sb[:, f, :],
                                start=(f == 0), stop=(f == FT - 1))
                        gt = wpool.tile([128, 256], F32, tag="ffgt")
                        nc.vector.tensor_mul(
                            out=gt, in0=yps,
                            in1=gateB[:, 256 * cc:256 * (cc + 1)])
                        nc.vector.tensor_add(
                            out=h_sb[:, u, 256 * cc:256 * (cc + 1)],
                            in0=h_sb[:, u, 256 * cc:256 * (cc + 1)], in1=gt)
                        nc.sync.dma_start(
                            out=bass.AP(tensor=h_out.tensor,
                                        offset=u * 128 * C + 256 * cc,
                                        ap=[[C, 128], [1, 256]]),
                            in_=h_sb[:, u, 256 * cc:256 * (cc + 1)])

        ff1_ctx.__exit__(None, None, None)

        ctx_pool.__exit__(None, None, None)

    nc.compile()
    return nc


_NC_CACHE = None


def _get_nc():
    global _NC_CACHE
    if _NC_CACHE is None:
        nc = bacc.Bacc("TRN2", target_bir_lowering=False, debug=False,
                       num_devices=NCORES)
        _NC_CACHE = _build(nc)
    return _NC_CACHE


def _bf(a):
    return np.ascontiguousarray(np.asarray(a, np.float32)).astype(
        ml_dtypes.bfloat16)


def kernel(**inputs) -> np.ndarray:
    h = np.asarray(inputs["hidden_states"], np.float32)[0]      # [S, C]
    enc = np.asarray(inputs["encoder_hidden_states"], np.float32)[0]
    emb = np.asarray(inputs["embedded_timestep"], np.float32)[0]  # [C]
    temb = np.asarray(inputs["temb"], np.float32)                # [1, 3C]
    cosT = np.ascontiguousarray(np.asarray(inputs["rope_cos"],
                                           np.float32).T)        # [D, S]
    sinT = np.ascontiguousarray(np.asarray(inputs["rope_sin"],
                                           np.float32).T)

    for k in ("attn1_qn", "attn1_kn", "attn2_qn", "attn2_kn"):
        assert np.allclose(np.asarray(inputs[k]), 1.0), f"{k} != ones"

    rot = np.zeros((D, D), np.float32)  # rot_out = rot_m.T @ q
    for d in range(D // 2):
        rot[64 + d, d] = -1.0
        rot[d, 64 + d] = 1.0

    aw2 = [np.asarray(inputs[f"a{i}_w2"], np.float32) for i in (1, 2, 3)]
    common = {
        "wq1": _bf(inputs["attn1_wq"]), "wk1": _bf(inputs["attn1_wk"]),
        "wv1": _bf(inputs["attn1_wv"]), "wo1": _bf(inputs["attn1_wo"]),
        "wq2": _bf(inputs["attn2_wq"]), "wo2": _bf(inputs["attn2_wo"]),
        "wk2": _bf(inputs["attn2_wk"]), "wv2": _bf(inputs["attn2_wv"]),
        "w1f": _bf(inputs["ff_w1"]), "w2f": _bf(inputs["ff_w2"]),
        "encT": _bf(enc.T),
        "aw1": _bf(np.stack([inputs["a1_w1"], inputs["a2_w1"],
                             inputs["a3_w1"]])),
        "emb_t": np.ascontiguousarray(emb.reshape(CT, 128).T),
        "rotm": round_fp32r(rot),
        "eye": round_fp32r(np.eye(128, dtype=np.float32)),
        "onec": np.ones((128, 1), np.float32),
        "onecb": np.ones((128, 1), np.float32).astype(ml_dtypes.bfloat16),
    }
    in_maps = []
    for r in range(NCORES):
        tok = slice(SL * r, SL * (r + 1))
        hd = slice(256 * r, 256 * (r + 1))
        m = dict(common)
        m["h_s"] = np.ascontiguousarray(h[tok, :])
        m["cosT"] = np.ascontiguousarray(cosT[:, tok])
        m["sinT"] = np.ascontiguousarray(sinT[:, tok])
        m["aw2l"] = _bf(np.stack([
            np.concatenate([a[:, 2048 * t + 256 * r:2048 * t + 256 * (r + 1)]
                            for t in range(3)], axis=1) for a in aw2]))
        m["temb_l"] = np.ascontiguousarray(np.stack(
            [temb[0, 2048 * t + 256 * r:2048 * t + 256 * (r + 1)]
             for t in range(3)]))
        in_maps.append(m)

    nc = _get_nc()
    res = run_bass_kernel_spmd(nc, in_maps, core_ids=list(range(NCORES)))
    out = np.concatenate([res.results[i]["h_out"] for i in range(NCORES)],
                         axis=0)
    return out.reshape(1, S, C).astype(np.float32)


if __name__ == "__main__":
    _get_nc()
    print("build + compile OK")


# revision 4
# speedup vs baseline: 1036.8432x; 1.0071x over previous
"""CosmosTransformerBlock on 8 Trainium2 NeuronCores — sequence-parallel.

Strategy v2 (sequence-parallel, replicated weights, bf16 matmuls):
- Each core owns SL=256 tokens of the residual stream; LayerNorm/AdaLN
  modulation, QKV, attention queries, wo, and the whole FF run locally on
  those tokens with full (replicated) weight matrices — identical FLOPs to
  tensor-parallel, but attn2 and FF need NO collectives at all.
- Self-attention needs full-sequence K/V: one fused AllGather of the local
  [K^T; V] buffer (bf16). Cross-attention K/V is computed head-sharded from
  the encoder (2 heads/core) and AllGather'd early (overlaps sublayer 0).
- AdaLN lora-up matmul is sharded 768 cols/core; the tiny e vectors are
  AllGather'd at kernel start.
- All matmuls run in bf16 (1 cyc/row on PE) with fp32 PSUM accumulation;
  residual h, LN stats, softmax normalization stay fp32.
"""

import sys

import numpy as np

try:
    import concourse.bass as bass
except ImportError:  # pragma: no cover
    sys.path.insert(0, "/opt/trn_rl_repo")
    import concourse.bass as bass

import ml_dtypes
import concourse.mybir as mybir
import concourse.tile as tile
from concourse import bacc
from concourse.bass_utils import run_bass_kernel_spmd

F32 = mybir.dt.float32
F32R = mybir.dt.float32r
BF16 = mybir.dt.bfloat16
AF = mybir.ActivationFunctionType
ALU = mybir.AluOpType

NCORES = 8
S, C, D, H = 2048, 2048, 128, 16
SL = S // NCORES      # 256 tokens per core
CT = C // 128         # 16 feature tiles
SC, CROSS, LORA, FF = 512, 1024, 256, 8192
FT = FF // 128        # 64 ff tiles
EPS = 1e-6
ISQD = float(D) ** -0.5

KV1K = H * D * SL     # 524288: K^T elems in kv1_in
KV1SZ = 2 * KV1K
KV2K = 2 * D * SC     # 131072: local-2-head K2^T elems in kv2_in
KV2SZ = 2 * KV2K
RG = [list(range(NCORES))]


def round_fp32r(a: np.ndarray) -> np.ndarray:
    b = np.ascontiguousarray(a, dtype=np.float32).view(np.uint32)
    lsb = (b >> np.uint32(12)) & np.uint32(1)
    r = (b + np.uint32(0x7FF) + lsb) & np.uint32(0xFFFFF000)
    return r.view(np.float32)


def _build(nc: bacc.Bacc):
    dram = lambda n, s, d: nc.dram_tensor(n, s, d, kind="ExternalInput").ap()

    h_in = dram("h_s", [SL, C], F32)
    wq1 = dram("wq1", [C, C], BF16)
    wk1 = dram("wk1", [C, C], BF16)
    wv1 = dram("wv1", [C, C], BF16)
    wo1 = dram("wo1", [C, C], BF16)
    wq2 = dram("wq2", [C, C], BF16)
    wk2 = dram("wk2", [CROSS, C], BF16)
    wv2 = dram("wv2", [CROSS, C], BF16)
    wo2 = dram("wo2", [C, C], BF16)
    w1f = dram("w1f", [C, FF], BF16)
    w2f = dram("w2f", [FF, C], BF16)
    encT = dram("encT", [CROSS, SC], BF16)
    aw1 = dram("aw1", [3, C, LORA], BF16)
    aw2l = dram("aw2l", [3, LORA, 768], BF16)
    temb_l = dram("temb_l", [3, 256], F32)
    emb_t = dram("emb_t", [128, CT], F32)
    cosT = dram("cosT", [D, SL], F32)
    sinT = dram("sinT", [D, SL], F32)
    rotm = dram("rotm", [D, D], F32R)
    eye = dram("eye", [128, 128], F32R)
    onec = dram("onec", [128, 1], F32R)
    onecb = dram("onecb", [128, 1], BF16)

    h_out = nc.dram_tensor("h_out", [SL, C], F32, kind="ExternalOutput").ap()

    e_in = nc.dram_tensor("e_in", [9, 256], F32).ap()
    e_out = nc.dram_tensor("e_out", [NCORES * 9, 256], F32,
                           addr_space="Shared").ap()
    kv2d = nc.dram_tensor("kv2d", [2 * 128 * H * SC], BF16).ap()
    kv1_in = nc.dram_tensor("kv1_in", [KV1SZ], BF16).ap()
    kv1_out = nc.dram_tensor("kv1_out", [NCORES * KV1SZ], BF16,
                             addr_space="Shared").ap()

    with tile.TileContext(nc) as tc:
        ctx_pool = tc.tile_pool(name="persist", bufs=1)
        persist = ctx_pool.__enter__()

        # AdaLN inputs lead the SP queue: the e-chain gates the K/V start
        # and hence the KV AllGather launch. h (LN) isn't needed until ~12us.
        emb_sb = persist.tile([128, CT], F32, tag="emb")
        nc.sync.dma_start(out=emb_sb, in_=emb_t)
        onecb_sb = persist.tile([128, 1], BF16, tag="onecb")
        nc.sync.dma_start(out=onecb_sb, in_=onecb)
        eps_sb = persist.tile([128, 1], F32, tag="eps")
        nc.vector.memset(eps_sb, EPS)
        h_sb = persist.tile([128, 2, C], F32, tag="h")
        eye_sb = persist.tile([128, 128], F32R, tag="eye")
        onec_sb = persist.tile([128, 1], F32R, tag="onec")
        rotm_sb = persist.tile([D, D], F32R, tag="rotm")
        cos2_sb = persist.tile([D, 2, SL], F32, tag="cos2")
        sin2_sb = persist.tile([D, 2, SL], F32, tag="sin2")
        gateB = persist.tile([128, C], F32, tag="gateB")
        q_sb = persist.tile([128, H, SL], BF16, tag="q")
        xnT_sb = persist.tile([128, CT, SL], BF16, tag="xnT")
        a_sb = persist.tile([128, H, SL], BF16, tag="a")

        # ---------------- AdaLN (sharded 768 cols/core) + e AllGather -------
        # All three sublayers' lora-down products y_L are computed first and
        # round-trip DRAM once; then the lora-up slices + temb -> e AllGather.
        with tc.tile_pool(name="adaln", bufs=3) as apool, \
             tc.tile_pool(name="adaln_ps", bufs=2, space="PSUM") as apsum:
            silu_sb = apool.tile([128, CT], BF16, tag="silu", bufs=1)
            nc.scalar.activation(out=silu_sb, in_=emb_sb, func=AF.Silu)
            temb_sb = apool.tile([1, 3, 256], F32, tag="tembl", bufs=1)
            nc.sync.dma_start(out=temb_sb, in_=bass.AP(
                tensor=temb_l.tensor, offset=0,
                ap=[[768, 1], [256, 3], [1, 256]]))
            e_loc = apool.tile([1, 9, 256], F32, tag="eloc", bufs=1)
            yall = apool.tile([1, 3, LORA], BF16, tag="yall", bufs=1)
            for L in range(3):
                aw1_sb = apool.tile([128, CT, LORA], BF16, tag="aw1")
                nc.sync.dma_start(
                    out=aw1_sb, in_=aw1[L].rearrange("(t p) m -> p t m",
                                                     p=128))
                y_ps = apsum.tile([1, LORA], F32, tag="yps")
                for ct in range(CT):
                    nc.tensor.matmul(y_ps, lhsT=silu_sb[:, ct:ct + 1],
                                     rhs=aw1_sb[:, ct, :],
                                     start=(ct == 0), stop=(ct == CT - 1))
                nc.scalar.copy(out=yall[:, L, :], in_=y_ps)
            # redistribute y to partitions via tiny PE transposes (no DRAM
            # round trip): y2[:, L, t2] = yall[0, L, 128*t2 : 128*(t2+1)]^T
            y2 = apool.tile([128, 3, 2], BF16, tag="y2", bufs=1)
            for L in range(3):
                for t2 in range(2):
                    y2p = apsum.tile([128, 1], F32, tag="y2p")
                    nc.tensor.matmul(
                        y2p, lhsT=yall[:, L, 128 * t2:128 * (t2 + 1)],
                        rhs=onecb_sb[0:1, :], start=True, stop=True)
                    nc.vector.tensor_copy(out=y2[:, L, t2:t2 + 1], in_=y2p)
            for L in range(3):
                aw2_sb = apool.tile([128, 2, 3, 256], BF16, tag="aw2")
                nc.sync.dma_start(
                    out=aw2_sb,
                    in_=aw2l[L].rearrange("(t p) (g n) -> p t g n",
                                          p=128, n=256))
                for g3 in range(3):
                    e_ps = apsum.tile([1, 256], F32, tag="eps2")
                    nc.tensor.matmul(e_ps, lhsT=y2[:, L, 0:1],
                                     rhs=aw2_sb[:, 0, g3, :],
                                     start=True, stop=False)
                    nc.tensor.matmul(e_ps, lhsT=y2[:, L, 1:2],
                                     rhs=aw2_sb[:, 1, g3, :],
                                     start=False, stop=True)
                    nc.vector.tensor_add(out=e_loc[:, 3 * L + g3, :],
                                         in0=e_ps,
                                         in1=temb_sb[:, g3, :])
            nc.sync.dma_start(out=bass.AP(
                tensor=e_in.tensor, offset=0,
                ap=[[2304, 1], [256, 9], [1, 256]]), in_=e_loc)
        nc.gpsimd.collective_compute(
            "AllGather", ALU.bypass, replica_groups=RG,
            ins=[e_in], outs=[e_out])

        nc.sync.dma_start(out=h_sb, in_=h_in.rearrange("(u p) c -> p u c",
                                                       p=128))
        nc.sync.dma_start(out=eye_sb, in_=eye)
        nc.sync.dma_start(out=onec_sb, in_=onec)
        nc.sync.dma_start(out=rotm_sb, in_=rotm)
        nc.sync.dma_start(out=cos2_sb, in_=bass.AP(
            tensor=cosT.tensor, offset=0, ap=[[SL, 128], [0, 2], [1, SL]]))
        nc.sync.dma_start(out=sin2_sb, in_=bass.AP(
            tensor=sinT.tensor, offset=0, ap=[[SL, 128], [0, 2], [1, SL]]))

        def _ebcast(L, t):
            # [128, 8, 256] broadcast view of e_out for (sublayer L, type t)
            return bass.AP(tensor=e_out.tensor, offset=(3 * L + t) * 256,
                           ap=[[0, 128], [9 * 256, 8], [1, 256]])

        # ---------------- modulate: LN + transpose + feature-major mod ------
        # LN and the transposes do NOT wait for the e AllGather; scale/shift
        # are applied per-partition (feature) on the transposed tiles.
        def modulate(L, mpool, mpsum):
            shT = mpool.tile([128, 2, 8], F32, tag="shT", bufs=1)
            opT = mpool.tile([128, 2, 8], F32, tag="opT", bufs=1)
            for par in range(2):
                nc.sync.dma_start(out=shT[:, par, :], in_=bass.AP(
                    tensor=e_out.tensor, offset=3 * L * 256 + par * 128,
                    ap=[[1, 128], [2304, 8]]))
                nc.sync.dma_start(out=opT[:, par, :], in_=bass.AP(
                    tensor=e_out.tensor, offset=(3 * L + 1) * 256 + par * 128,
                    ap=[[1, 128], [2304, 8]]))
            nc.vector.tensor_scalar_add(out=opT, in0=opT, scalar1=1.0)
            nc.sync.dma_start(out=gateB, in_=_ebcast(L, 2))
            for u in range(2):
                x = h_sb[:, u, :]
                stats = mpool.tile([128, 4, 6], F32, tag="stats")
                xv = x.rearrange("p (g n) -> p g n", n=512)
                for g in range(4):
                    nc.vector.bn_stats(out=stats[:, g, :], in_=xv[:, g, :])
                mv = mpool.tile([128, 2], F32, tag="mv")
                nc.vector.bn_aggr(out=mv, in_=stats)
                sd = mpool.tile([128, 1], F32, tag="sd")
                nc.scalar.activation(out=sd, in_=mv[:, 1:2], func=AF.Sqrt,
                                     bias=eps_sb)
                rstd = mpool.tile([128, 1], F32, tag="rstd")
                nc.vector.reciprocal(out=rstd, in_=sd)
                t1 = mpool.tile([128, C], F32R, tag="mod_t1",
                                name=f"t1_{u}")
                nc.vector.tensor_scalar(out=t1, in0=x, scalar1=mv[:, 0:1],
                                        scalar2=rstd, op0=ALU.subtract,
                                        op1=ALU.mult)
                for ct in range(CT):
                    tp = mpsum.tile([128, 128], F32R, tag="tpsum")
                    nc.tensor.transpose(
                        tp, t1[:, 128 * ct:128 * (ct + 1)], eye_sb)
                    nc.vector.tensor_scalar(
                        out=xnT_sb[:, ct, 128 * u:128 * (u + 1)], in0=tp,
                        scalar1=opT[:, ct % 2, ct // 2:ct // 2 + 1],
                        scalar2=shT[:, ct % 2, ct // 2:ct // 2 + 1],
                        op0=ALU.mult, op1=ALU.add)

        # batched rms(+rope) epilogue: src sbuf f32r [128, 2, 256] (2 heads)
        def rms_ep(src, dst, rope, pool, psum):
            sq = pool.tile([128, 2 * SL], F32R, tag="ep_sq")
            nc.scalar.activation(out=sq, in_=src, func=AF.Square)
            ss = psum.tile([1, 2 * SL], F32, tag="ep_ss")
            nc.tensor.matmul(ss, lhsT=onec_sb, rhs=sq, start=True, stop=True)
            sd = pool.tile([1, 2 * SL], F32, tag="ep_sd")
            nc.scalar.activation(out=sd, in_=ss, func=AF.Sqrt, scale=1.0 / D,
                                 bias=eps_sb[0:1, :])
            rc = pool.tile([1, 2 * SL], F32, tag="ep_rc")
            nc.vector.reciprocal(out=rc, in_=sd)
            rb = pool.tile([128, 2 * SL], F32, tag="ep_rb")
            nc.gpsimd.partition_broadcast(out_ap=rb, in_ap=rc)
            if rope:
                rot = psum.tile([128, 2 * SL], F32, tag="ep_rot")
                nc.tensor.matmul(rot, lhsT=rotm_sb, rhs=src,
                                 start=True, stop=True)
                t1 = pool.tile([128, 2 * SL], F32, tag="ep_t1")
                nc.vector.tensor_mul(out=t1, in0=src, in1=cos2_sb)
                t2 = pool.tile([128, 2 * SL], F32, tag="ep_t2")
                nc.vector.tensor_mul(out=t2, in0=rot, in1=sin2_sb)
                nc.vector.tensor_add(out=t1, in0=t1, in1=t2)
                nc.vector.tensor_mul(out=dst, in0=t1, in1=rb)
            else:
                nc.vector.tensor_mul(out=dst, in0=src, in1=rb)

        # wo + gated residual (shared by attn1/attn2); first chunks may
        # be prefetched into `pre` while the preceding phase runs
        def wo_prefetch(wX, pool, n=2):
            wv_ = wX.rearrange("(t p) (cc m) -> p cc t m", p=128, m=512)
            pre = []
            for cc in range(n):
                wo_sb = pool.tile([128, CT, 512], BF16, tag="wos")
                nc.sync.dma_start(out=wo_sb, in_=wv_[:, cc])
                pre.append(wo_sb)
            return wv_, pre

        def wo_residual(wv_, pool, pre):
            with tc.tile_pool(name="wops", bufs=2, space="PSUM") as psum:
                for cc in range(4):
                    if cc < len(pre):
                        wo_sb = pre[cc]
                    else:
                        wo_sb = pool.tile([128, CT, 512], BF16, tag="wos")
                        nc.sync.dma_start(out=wo_sb, in_=wv_[:, cc])
                    for u in range(2):
                        ops = psum.tile([128, 512], F32, tag="wops")
                        for hh in range(H):
                            nc.tensor.matmul(
                                ops,
                                lhsT=a_sb[:, hh, 128 * u:128 * (u + 1)],
                                rhs=wo_sb[:, hh, :],
                                start=(hh == 0), stop=(hh == H - 1))
                        gt = pool.tile([128, 512], F32, tag="wogt")
                        nc.vector.tensor_mul(
                            out=gt, in0=ops,
                            in1=gateB[:, 512 * cc:512 * (cc + 1)])
                        nc.vector.tensor_add(
                            out=h_sb[:, u, 512 * cc:512 * (cc + 1)],
                            in0=h_sb[:, u, 512 * cc:512 * (cc + 1)], in1=gt)

        # ======================= sublayer 0: self attention ================
        with tc.tile_pool(name="mod0", bufs=2) as mpool, \
             tc.tile_pool(name="mod0ps", bufs=2, space="PSUM") as mpsum:
            modulate(0, mpool, mpsum)

        # K/V in 4 head-groups of 4, written to k1_in/v1_in, then AllGathers
        wq1v = wq1.rearrange("(t p) (g m) -> p g t m", p=128, m=512)
        wk1v = wk1.rearrange("(t p) (g m) -> p g t m", p=128, m=512)
        wv1v = wv1.rearrange("(t p) (g m) -> p g t m", p=128, m=512)
        with tc.tile_pool(name="kv1w", bufs=2) as wpool, \
             tc.tile_pool(name="kv1e", bufs=1) as epool, \
             tc.tile_pool(name="kv1ps", bufs=1, space="PSUM") as qpsum, \
             tc.tile_pool(name="kv1ps2", bufs=1, space="PSUM") as qpsum2:
            # K pass first: the long rms+rope epilogue chains then overlap
            # the V matmul pass, so the AllGather is gated only by V's short
            # eviction chain.
            for g in range(4):
                wk_sb = wpool.tile([128, CT, 512], BF16, tag="wk")
                nc.sync.dma_start(out=wk_sb, in_=wk1v[:, g])
                pk = qpsum.tile([128, 4, SL], F32, tag="pk", bufs=2)
                for ct in range(CT):
                    st, sp = (ct == 0), (ct == CT - 1)
                    for j in range(4):
                        nc.tensor.matmul(
                            pk[:, j, :],
                            lhsT=wk_sb[:, ct, 128 * j:128 * (j + 1)],
                            rhs=xnT_sb[:, ct, :], start=st, stop=sp)
                kf = epool.tile([128, 4, SL], F32R, tag="kf")
                nc.vector.tensor_copy(out=kf, in_=pk)
                for jj in range(2):
                    ke = epool.tile([128, 2, SL], BF16, tag="ke")
                    rms_ep(kf[:, 2 * jj:2 * jj + 2, :], ke, True,
                           epool, qpsum2)
                    nc.sync.dma_start(
                        out=bass.AP(
                            tensor=kv1_in.tensor,
                            offset=(4 * g + 2 * jj) * D * SL,
                            ap=[[SL, 128], [D * SL, 2], [1, SL]]),
                        in_=ke)
            for g in range(4):
                wv_sb = wpool.tile([128, CT, 512], BF16, tag="wv")
                nc.sync.dma_start(out=wv_sb, in_=wv1v[:, g])
                pv = qpsum.tile([128, 2, 512], F32, tag="pv")
                for ct in range(CT):
                    st, sp = (ct == 0), (ct == CT - 1)
                    for u in range(2):
                        nc.tensor.matmul(
                            pv[:, u, :],
                            lhsT=xnT_sb[:, ct, 128 * u:128 * (u + 1)],
                            rhs=wv_sb[:, ct, :], start=st, stop=sp)
                ve = epool.tile([128, 2, 512], BF16, tag="ve")
                nc.vector.tensor_copy(out=ve, in_=pv)
                nc.sync.dma_start(
                    out=bass.AP(tensor=kv1_in.tensor,
                                offset=KV1K + 512 * g,
                                ap=[[C, 128], [128 * C, 2], [1, 512]]),
                    in_=ve)
        nc.gpsimd.collective_compute(
            "AllGather", ALU.bypass, replica_groups=RG,
            ins=[kv1_in], outs=[kv1_out])

        # ---- during the AllGather window: Q (rms+rope) and enc K2/V2 ------
        with tc.tile_pool(name="q1w", bufs=2) as wpool, \
             tc.tile_pool(name="q1e", bufs=1) as epool, \
             tc.tile_pool(name="q1ps", bufs=1, space="PSUM") as qpsum, \
             tc.tile_pool(name="q1ps2", bufs=1, space="PSUM") as qpsum2:
            for g in range(4):
                wq_sb = wpool.tile([128, CT, 512], BF16, tag="wq")
                nc.scalar.dma_start(out=wq_sb, in_=wq1v[:, g])
                pq = qpsum.tile([128, 4, SL], F32, tag="pq")
                for ct in range(CT):
                    st, sp = (ct == 0), (ct == CT - 1)
                    for j in range(4):
                        nc.tensor.matmul(
                            pq[:, j, :],
                            lhsT=wq_sb[:, ct, 128 * j:128 * (j + 1)],
                            rhs=xnT_sb[:, ct, :], start=st, stop=sp)
                qf = epool.tile([128, 4, SL], F32R, tag="qf")
                nc.vector.tensor_copy(out=qf, in_=pq)
                for jj in range(2):
                    h0 = 4 * g + 2 * jj
                    rms_ep(qf[:, 2 * jj:2 * jj + 2, :],
                           q_sb[:, h0:h0 + 2, :], True, epool, qpsum2)

        # replicated cross-attn K2^T/V2 from the encoder (all 16 heads);
        # results are spilled to DRAM and reloaded at sdpa2 to keep SBUF free
        wk2v = wk2.rearrange("(t p) (g m) -> p g t m", p=128, m=256)
        wv2v = wv2.rearrange("(t p) (g m) -> p g t m", p=128, m=256)
        with tc.tile_pool(name="enckv", bufs=2) as kpool, \
             tc.tile_pool(name="enckvps", bufs=1, space="PSUM") as kpsum, \
             tc.tile_pool(name="enckvps2", bufs=1, space="PSUM") as kpsum2:
            k2_sb = kpool.tile([128, H, SC], BF16, tag="k2", bufs=1)
            v2_sb = kpool.tile([128, 4, H, D], BF16, tag="v2", bufs=1)
            enc_sb = kpool.tile([128, 8, SC], BF16, tag="enc", bufs=1)
            nc.scalar.dma_start(out=enc_sb,
                              in_=encT.rearrange("(t p) s -> p t s", p=128))
            for g in range(8):  # pairs of heads
                wk2_sb = kpool.tile([128, 8, 256], BF16, tag="wk2")
                nc.scalar.dma_start(out=wk2_sb, in_=wk2v[:, g])
                wv2_sb = kpool.tile([128, 8, 256], BF16, tag="wv2")
                nc.scalar.dma_start(out=wv2_sb, in_=wv2v[:, g])
                pk2 = kpsum.tile([128, 2, SC], F32, tag="pk2")
                pv2 = kpsum.tile([128, 4, 256], F32, tag="pv2")
                for t in range(8):
                    st, sp = (t == 0), (t == 7)
                    for j in range(2):
                        nc.tensor.matmul(
                            pk2[:, j, :],
                            lhsT=wk2_sb[:, t, 128 * j:128 * (j + 1)],
                            rhs=enc_sb[:, t, :], start=st, stop=sp)
                    for tt in range(4):
                        nc.tensor.matmul(
                            pv2[:, tt, :],
                            lhsT=enc_sb[:, t, 128 * tt:128 * (tt + 1)],
                            rhs=wv2_sb[:, t, :], start=st, stop=sp)
                kf2 = kpool.tile([128, 2, SC], F32R, tag="kf2")
                nc.vector.tensor_copy(out=kf2, in_=pk2)
                # batched rms over head dim (partition), no rope
                sq = kpool.tile([128, 2, SC], F32R, tag="k2sq")
                nc.scalar.activation(out=sq, in_=kf2, func=AF.Square)
                ss = kpsum2.tile([1, 2, SC], F32, tag="k2ss")
                for j in range(2):
                    nc.tensor.matmul(ss[:, j, :], lhsT=onec_sb,
                                     rhs=sq[:, j, :], start=True, stop=True)
                sd = kpool.tile([1, 2 * SC], F32, tag="k2sd")
                nc.scalar.activation(out=sd, in_=ss, func=AF.Sqrt,
                                     scale=1.0 / D, bias=eps_sb[0:1, :])
                rc = kpool.tile([1, 2 * SC], F32, tag="k2rc")
                nc.vector.reciprocal(out=rc, in_=sd)
                rb = kpool.tile([128, 2 * SC], F32, tag="k2rb")
                nc.gpsimd.partition_broadcast(out_ap=rb, in_ap=rc)
                nc.vector.tensor_mul(out=k2_sb[:, 2 * g:2 * g + 2, :],
                                     in0=kf2, in1=rb)
                nc.vector.tensor_copy(out=v2_sb[:, :, 2 * g:2 * g + 2, :],
                                      in_=pv2)
            nc.sync.dma_start(out=bass.AP(
                tensor=kv2d.tensor, offset=0,
                ap=[[H * SC, 128], [1, H * SC]]), in_=k2_sb)
            nc.sync.dma_start(out=bass.AP(
                tensor=kv2d.tensor, offset=128 * H * SC,
                ap=[[H * SC, 128], [1, H * SC]]), in_=v2_sb)

        # sdpa over 2 halves of 8 heads; 4-head interleaved softmax rounds.
        # PE emission is software-pipelined: scores(kt) are issued before
        # den/av(kt-1) so PE never stalls on the Exp of the current round.
        def sdpa1(spool, spsum, spsum2):
            for quarter in range(4):
                hb0 = 4 * quarter
                k_sb = spool.tile([128, 4, 8, SL], BF16, tag="ksb", bufs=2)
                v_sb = spool.tile([128, 16, 512], BF16, tag="vsb", bufs=2)
                for r in range(NCORES):
                    nc.sync.dma_start(out=k_sb[:, :, r, :], in_=bass.AP(
                        tensor=kv1_out.tensor,
                        offset=hb0 * D * SL + r * KV1SZ,
                        ap=[[SL, 128], [D * SL, 4], [1, SL]]))
                    nc.sync.dma_start(out=v_sb[:, 2 * r:2 * r + 2, :],
                                      in_=bass.AP(
                        tensor=kv1_out.tensor,
                        offset=KV1K + hb0 * D + r * KV1SZ,
                        ap=[[C, 128], [128 * C, 2], [1, 512]]))
                if True:
                    hb = 0
                    av = spsum.tile([128, 4, SL], F32, tag="av")
                    den = spsum.tile([1, 2, 2 * SL], F32, tag="den")

                    def sc_round(kt):
                        sps = spsum2.tile([128, 4, SL], F32, tag="sps",
                                          name=f"sps{kt}")
                        for j in range(4):
                            nc.tensor.matmul(
                                sps[:, j, :],
                                lhsT=k_sb[:, j, kt // 2,
                                          128 * (kt % 2):128 * (kt % 2) + 128],
                                rhs=q_sb[:, hb0 + j, :],
                                start=True, stop=True)
                        pt = spool.tile([128, 4, SL], BF16, tag="pt",
                                        name=f"pt{kt}")
                        nc.scalar.activation(out=pt, in_=sps, func=AF.Exp,
                                             scale=ISQD)
                        return pt

                    def av_round(kt, pt):
                        st, sp = (kt == 0), (kt == 15)
                        for dh in range(2):
                            nc.tensor.matmul(
                                den[:, dh, :], lhsT=onecb_sb,
                                rhs=pt[:, 2 * dh:2 * dh + 2, :],
                                start=st, stop=sp)
                        for j in range(4):
                            nc.tensor.matmul(
                                av[:, j, :],
                                lhsT=v_sb[:, kt, 128 * j:128 * (j + 1)],
                                rhs=pt[:, j, :], start=st, stop=sp)

                    prev = sc_round(0)
                    for kt in range(1, 16):
                        cur = sc_round(kt)
                        av_round(kt - 1, prev)
                        prev = cur
                    av_round(15, prev)
                    rc = spool.tile([1, 4 * SL], F32, tag="sd_rc")
                    nc.vector.reciprocal(out=rc, in_=den)
                    rb = spool.tile([128, 4 * SL], F32, tag="sd_rb")
                    nc.gpsimd.partition_broadcast(out_ap=rb, in_ap=rc)
                    nc.vector.tensor_mul(
                        out=a_sb[:, hb0:hb0 + 4, :],
                        in0=av, in1=rb)

        wo1_ctx = tc.tile_pool(name="wo1", bufs=2)
        wo1_pool = wo1_ctx.__enter__()
        wo1v, wo1_pre = wo_prefetch(wo1, wo1_pool)

        with tc.tile_pool(name="sdpa1", bufs=3) as spool, \
             tc.tile_pool(name="sdpa1ps", bufs=1, space="PSUM") as spsum, \
             tc.tile_pool(name="sdpa1ps2", bufs=2, space="PSUM") as spsum2:
            sdpa1(spool, spsum, spsum2)

        wo_residual(wo1v, wo1_pool, wo1_pre)
        wo1_ctx.__exit__(None, None, None)

        # ======================= sublayer 1: cross attention ===============
        with tc.tile_pool(name="mod1", bufs=2) as mpool, \
             tc.tile_pool(name="mod1ps", bufs=2, space="PSUM") as mpsum:
            modulate(1, mpool, mpsum)

        wq2v = wq2.rearrange("(t p) (g m) -> p g t m", p=128, m=512)
        with tc.tile_pool(name="q2w", bufs=2) as wpool, \
             tc.tile_pool(name="q2e", bufs=1) as epool, \
             tc.tile_pool(name="q2ps", bufs=1, space="PSUM") as qpsum, \
             tc.tile_pool(name="q2ps2", bufs=1, space="PSUM") as qpsum2:
            for g in range(4):
                wq_sb = wpool.tile([128, CT, 512], BF16, tag="wq2")
                nc.scalar.dma_start(out=wq_sb, in_=wq2v[:, g])
                pq = qpsum.tile([128, 4, SL], F32, tag="pq2")
                for ct in range(CT):
                    st, sp = (ct == 0), (ct == CT - 1)
                    for j in range(4):
                        nc.tensor.matmul(
                            pq[:, j, :],
                            lhsT=wq_sb[:, ct, 128 * j:128 * (j + 1)],
                            rhs=xnT_sb[:, ct, :], start=st, stop=sp)
                qf = epool.tile([128, 4, SL], F32R, tag="qf2")
                nc.vector.tensor_copy(out=qf, in_=pq)
                for jj in range(2):
                    h0 = 4 * g + 2 * jj
                    rms_ep(qf[:, 2 * jj:2 * jj + 2, :],
                           q_sb[:, h0:h0 + 2, :], False, epool, qpsum2)

        def sdpa2(spool, spsum, spsum2):
            k2_sb = spool.tile([128, H, SC], BF16, tag="k2r", bufs=1)
            nc.sync.dma_start(out=k2_sb, in_=bass.AP(
                tensor=kv2d.tensor, offset=0,
                ap=[[H * SC, 128], [1, H * SC]]))
            v2_sb = spool.tile([128, 4, H, D], BF16, tag="v2r", bufs=1)
            nc.sync.dma_start(out=v2_sb, in_=bass.AP(
                tensor=kv2d.tensor, offset=128 * H * SC,
                ap=[[H * SC, 128], [1, H * SC]]))
            for sg in range(4):
                hb = 4 * sg
                av = spsum.tile([128, 4, SL], F32, tag="av2")
                den = spsum.tile([1, 2, 2 * SL], F32, tag="den2")

                def sc_round(kt):
                    sps = spsum2.tile([128, 4, SL], F32, tag="sps2",
                                      name=f"sps2_{kt}")
                    for j in range(4):
                        nc.tensor.matmul(
                            sps[:, j, :],
                            lhsT=k2_sb[:, hb + j, 128 * kt:128 * (kt + 1)],
                            rhs=q_sb[:, hb + j, :], start=True, stop=True)
                    pt = spool.tile([128, 4, SL], BF16, tag="pt2",
                                    name=f"pt2_{kt}")
                    nc.scalar.activation(out=pt, in_=sps, func=AF.Exp,
                                         scale=ISQD)
                    return pt

                def av_round(kt, pt):
                    st, sp = (kt == 0), (kt == 3)
                    for dh in range(2):
                        nc.tensor.matmul(
                            den[:, dh, :], lhsT=onecb_sb,
                            rhs=pt[:, 2 * dh:2 * dh + 2, :],
                            start=st, stop=sp)
                    for j in range(4):
                        nc.tensor.matmul(
                            av[:, j, :],
                            lhsT=v2_sb[:, kt, hb + j, :],
                            rhs=pt[:, j, :], start=st, stop=sp)

                prev = sc_round(0)
                for kt in range(1, 4):
                    cur = sc_round(kt)
                    av_round(kt - 1, prev)
                    prev = cur
                av_round(3, prev)
                rc = spool.tile([1, 4 * SL], F32, tag="sd2_rc")
                nc.vector.reciprocal(out=rc, in_=den)
                rb = spool.tile([128, 4 * SL], F32, tag="sd2_rb")
                nc.gpsimd.partition_broadcast(out_ap=rb, in_ap=rc)
                nc.vector.tensor_mul(out=a_sb[:, hb:hb + 4, :],
                                     in0=av, in1=rb)

        w1v = w1f.rearrange("(t p) (fg m) -> p fg t m", p=128, m=512)
        ff1_ctx = tc.tile_pool(name="ff1", bufs=2)
        ff1_pool = ff1_ctx.__enter__()
        wo2_ctx = tc.tile_pool(name="wo2", bufs=2)
        wo2_pool = wo2_ctx.__enter__()
        wo2v, wo2_pre = wo_prefetch(wo2, wo2_pool)
        w1_pre = []
        for fg in range(2):
            w1_sb = ff1_pool.tile([128, CT, 512], BF16, tag="w1")
            nc.sync.dma_start(out=w1_sb, in_=w1v[:, fg])
            w1_pre.append(w1_sb)

        with tc.tile_pool(name="sdpa2", bufs=3) as spool, \
             tc.tile_pool(name="sdpa2ps", bufs=1, space="PSUM") as spsum, \
             tc.tile_pool(name="sdpa2ps2", bufs=2, space="PSUM") as spsum2:
            sdpa2(spool, spsum, spsum2)

        wo_residual(wo2v, wo2_pool, wo2_pre)
        wo2_ctx.__exit__(None, None, None)

        # ======================= sublayer 2: feed forward ==================
        with tc.tile_pool(name="mod2", bufs=2) as mpool, \
             tc.tile_pool(name="mod2ps", bufs=2, space="PSUM") as mpsum:
            modulate(2, mpool, mpsum)

        w2v = w2f.rearrange("(f p) (cc m) -> p cc f m", p=128, m=256)
        with tc.tile_pool(name="ffg", bufs=1) as gpool:
            g_sb = gpool.tile([128, FT, SL], BF16, tag="g")
            with tc.tile_pool(name="ff1ps", bufs=4, space="PSUM") as fpsum:
                for fg in range(16):
                    if fg < len(w1_pre):
                        w1_sb = w1_pre[fg]
                    else:
                        w1_sb = ff1_pool.tile([128, CT, 512], BF16, tag="w1")
                        nc.sync.dma_start(out=w1_sb, in_=w1v[:, fg])
                    for ft in range(4):
                        gps = fpsum.tile([128, SL], F32, tag="gps")
                        for ct in range(CT):
                            nc.tensor.matmul(
                                gps,
                                lhsT=w1_sb[:, ct, 128 * ft:128 * (ft + 1)],
                                rhs=xnT_sb[:, ct, :],
                                start=(ct == 0), stop=(ct == CT - 1))
                        nc.scalar.activation(
                            out=g_sb[:, 4 * fg + ft, :], in_=gps,
                            func=AF.Gelu)
            with tc.tile_pool(name="ff2", bufs=2) as wpool, \
                 tc.tile_pool(name="ff2ps", bufs=2, space="PSUM") as fpsum:
                for cc in range(8):
                    w2_sb = wpool.tile([128, FT, 256], BF16, tag="w2")
                    nc.sync.dma_start(out=w2_sb, in_=w2v[:, cc])
                    for u in range(2):
                        yps = fpsum.tile([128, 256], F32, tag="yps")
                        for f in range(FT):
                            nc.tensor.matmul(
                                yps,
                                lhsT=g_sb[:, f, 128 * u:128 * (u + 1)],
                                rhs=w2_sb[:, f, :],
                                start=(f == 0), stop=(f == FT - 1))
                        gt = wpool.tile([128, 256], F32, tag="ffgt")
                        nc.vector.tensor_mul(
                            out=gt, in0=yps,
                            in1=gateB[:, 256 * cc:256 * (cc + 1)])
                        nc.vector.tensor_add(
                            out=h_sb[:, u, 256 * cc:256 * (cc + 1)],
                            in0=h_sb[:, u, 256 * cc:256 * (cc + 1)], in1=gt)
                        nc.sync.dma_start(
                            out=bass.AP(tensor=h_out.tensor,
                                        offset=u * 128 * C + 256 * cc,
                                        ap=[[C, 128], [1, 256]]),
                            in_=h_sb[:, u, 256 * cc:256 * (cc + 1)])

        ff1_ctx.__exit__(None, None, None)

        ctx_pool.__exit__(None, None, None)

    nc.compile()
    return nc


_NC_CACHE = None


def _get_nc():
    global _NC_CACHE
    if _NC_CACHE is None:
        nc = bacc.Bacc("TRN2", target_bir_lowering=False, debug=False,
                       num_devices=NCORES)
        _NC_CACHE = _build(nc)
    return _NC_CACHE


def _bf(a):
    return np.ascontiguousarray(np.asarray(a, np.float32)).astype(
        ml_dtypes.bfloat16)


def kernel(**inputs) -> np.ndarray:
    h = np.asarray(inputs["hidden_states"], np.float32)[0]      # [S, C]
    enc = np.asarray(inputs["encoder_hidden_states"], np.float32)[0]
    emb = np.asarray(inputs["embedded_timestep"], np.float32)[0]  # [C]
    temb = np.asarray(inputs["temb"], np.float32)                # [1, 3C]
    cosT = np.ascontiguousarray(np.asarray(inputs["rope_cos"],
                                           np.float32).T)        # [D, S]
    sinT = np.ascontiguousarray(np.asarray(inputs["rope_sin"],
                                           np.float32).T)

    for k in ("attn1_qn", "attn1_kn", "attn2_qn", "attn2_kn"):
        assert np.allclose(np.asarray(inputs[k]), 1.0), f"{k} != ones"

    rot = np.zeros((D, D), np.float32)  # rot_out = rot_m.T @ q
    for d in range(D // 2):
        rot[64 + d, d] = -1.0
        rot[d, 64 + d] = 1.0

    aw2 = [np.asarray(inputs[f"a{i}_w2"], np.float32) for i in (1, 2, 3)]
    common = {
        "wq1": _bf(inputs["attn1_wq"]), "wk1": _bf(inputs["attn1_wk"]),
        "wv1": _bf(inputs["attn1_wv"]), "wo1": _bf(inputs["attn1_wo"]),
        "wq2": _bf(inputs["attn2_wq"]), "wo2": _bf(inputs["attn2_wo"]),
        "wk2": _bf(inputs["attn2_wk"]), "wv2": _bf(inputs["attn2_wv"]),
        "w1f": _bf(inputs["ff_w1"]), "w2f": _bf(inputs["ff_w2"]),
        "encT": _bf(enc.T),
        "aw1": _bf(np.stack([inputs["a1_w1"], inputs["a2_w1"],
                             inputs["a3_w1"]])),
        "emb_t": np.ascontiguousarray(emb.reshape(CT, 128).T),
        "rotm": round_fp32r(rot),
        "eye": round_fp32r(np.eye(128, dtype=np.float32)),
        "onec": np.ones((128, 1), np.float32),
        "onecb": np.ones((128, 1), np.float32).astype(ml_dtypes.bfloat16),
    }
    in_maps = []
    for r in range(NCORES):
        tok = slice(SL * r, SL * (r + 1))
        hd = slice(256 * r, 256 * (r + 1))
        m = dict(common)
        m["h_s"] = np.ascontiguousarray(h[tok, :])
        m["cosT"] = np.ascontiguousarray(cosT[:, tok])
        m["sinT"] = np.ascontiguousarray(sinT[:, tok])
        m["aw2l"] = _bf(np.stack([
            np.concatenate([a[:, 2048 * t + 256 * r:2048 * t + 256 * (r + 1)]
                            for t in range(3)], axis=1) for a in aw2]))
        m["temb_l"] = np.ascontiguousarray(np.stack(
            [temb[0, 2048 * t + 256 * r:2048 * t + 256 * (r + 1)]
             for t in range(3)]))
        in_maps.append(m)

    nc = _get_nc()
    res = run_bass_kernel_spmd(nc, in_maps, core_ids=list(range(NCORES)))
    out = np.concatenate([res.results[i]["h_out"] for i in range(NCORES)],
                         axis=0)
    return out.reshape(1, S, C).astype(np.float32)


if __name__ == "__main__":
    _get_nc()
    print("build + compile OK")


# revision 5
# speedup vs baseline: 1040.8817x; 1.0039x over previous
"""CosmosTransformerBlock on 8 Trainium2 NeuronCores — sequence-parallel.

Strategy v2 (sequence-parallel, replicated weights, bf16 matmuls):
- Each core owns SL=256 tokens of the residual stream; LayerNorm/AdaLN
  modulation, QKV, attention queries, wo, and the whole FF run locally on
  those tokens with full (replicated) weight matrices — identical FLOPs to
  tensor-parallel, but attn2 and FF need NO collectives at all.
- Self-attention needs full-sequence K/V: one fused AllGather of the local
  [K^T; V] buffer (bf16). Cross-attention K/V is computed head-sharded from
  the encoder (2 heads/core) and AllGather'd early (overlaps sublayer 0).
- AdaLN lora-up matmul is sharded 768 cols/core; the tiny e vectors are
  AllGather'd at kernel start.
- All matmuls run in bf16 (1 cyc/row on PE) with fp32 PSUM accumulation;
  residual h, LN stats, softmax normalization stay fp32.
"""

import sys

import numpy as np

try:
    import concourse.bass as bass
except ImportError:  # pragma: no cover
    sys.path.insert(0, "/opt/trn_rl_repo")
    import concourse.bass as bass

import ml_dtypes
import concourse.mybir as mybir
import concourse.tile as tile
from concourse import bacc
from concourse.bass_utils import run_bass_kernel_spmd

F32 = mybir.dt.float32
F32R = mybir.dt.float32r
BF16 = mybir.dt.bfloat16
AF = mybir.ActivationFunctionType
ALU = mybir.AluOpType

NCORES = 8
S, C, D, H = 2048, 2048, 128, 16
SL = S // NCORES      # 256 tokens per core
CT = C // 128         # 16 feature tiles
SC, CROSS, LORA, FF = 512, 1024, 256, 8192
FT = FF // 128        # 64 ff tiles
EPS = 1e-6
ISQD = float(D) ** -0.5

KV1K = H * D * SL     # 524288: K^T elems in kv1_in
KV1SZ = 2 * KV1K
KV2K = 2 * D * SC     # 131072: local-2-head K2^T elems in kv2_in
KV2SZ = 2 * KV2K
RG = [list(range(NCORES))]


def round_fp32r(a: np.ndarray) -> np.ndarray:
    b = np.ascontiguousarray(a, dtype=np.float32).view(np.uint32)
    lsb = (b >> np.uint32(12)) & np.uint32(1)
    r = (b + np.uint32(0x7FF) + lsb) & np.uint32(0xFFFFF000)
    return r.view(np.float32)


def _build(nc: bacc.Bacc):
    dram = lambda n, s, d: nc.dram_tensor(n, s, d, kind="ExternalInput").ap()

    h_in = dram("h_s", [SL, C], F32)
    wq1 = dram("wq1", [C, C], BF16)
    wk1 = dram("wk1", [C, C], BF16)
    wv1 = dram("wv1", [C, C], BF16)
    wo1 = dram("wo1", [C, C], BF16)
    wq2 = dram("wq2", [C, C], BF16)
    wk2 = dram("wk2", [CROSS, C], BF16)
    wv2 = dram("wv2", [CROSS, C], BF16)
    wo2 = dram("wo2", [C, C], BF16)
    w1f = dram("w1f", [C, FF], BF16)
    w2f = dram("w2f", [FF, C], BF16)
    encT = dram("encT", [CROSS, SC], BF16)
    aw1 = dram("aw1", [3, C, LORA], BF16)
    aw2l = dram("aw2l", [3, LORA, 768], BF16)
    temb_l = dram("temb_l", [3, 256], F32)
    emb_t = dram("emb_t", [128, CT], F32)
    cosT = dram("cosT", [D, SL], F32)
    sinT = dram("sinT", [D, SL], F32)
    rotm = dram("rotm", [D, D], F32R)
    eye = dram("eye", [128, 128], F32R)
    onec = dram("onec", [128, 1], F32R)
    onecb = dram("onecb", [128, 1], BF16)

    h_out = nc.dram_tensor("h_out", [SL, C], F32, kind="ExternalOutput").ap()

    e_in = nc.dram_tensor("e_in", [9, 256], F32).ap()
    e_out = nc.dram_tensor("e_out", [NCORES * 9, 256], F32,
                           addr_space="Shared").ap()
    kv2d = nc.dram_tensor("kv2d", [2 * 128 * H * SC], BF16).ap()
    kv1_in = nc.dram_tensor("kv1_in", [KV1SZ], BF16).ap()
    kv1_out = nc.dram_tensor("kv1_out", [NCORES * KV1SZ], BF16,
                             addr_space="Shared").ap()

    with tile.TileContext(nc) as tc:
        ctx_pool = tc.tile_pool(name="persist", bufs=1)
        persist = ctx_pool.__enter__()

        # AdaLN inputs lead the SP queue: the e-chain gates the K/V start
        # and hence the KV AllGather launch. h (LN) isn't needed until ~12us.
        emb_sb = persist.tile([128, CT], F32, tag="emb")
        nc.sync.dma_start(out=emb_sb, in_=emb_t)
        onecb_sb = persist.tile([128, 1], BF16, tag="onecb")
        nc.sync.dma_start(out=onecb_sb, in_=onecb)
        eps_sb = persist.tile([128, 1], F32, tag="eps")
        nc.vector.memset(eps_sb, EPS)
        h_sb = persist.tile([128, 2, C], F32, tag="h")
        eye_sb = persist.tile([128, 128], F32R, tag="eye")
        onec_sb = persist.tile([128, 1], F32R, tag="onec")
        rotm_sb = persist.tile([D, D], F32R, tag="rotm")
        cos2_sb = persist.tile([D, 2, SL], F32, tag="cos2")
        sin2_sb = persist.tile([D, 2, SL], F32, tag="sin2")
        gateB = persist.tile([128, C], F32, tag="gateB")
        q_sb = persist.tile([128, H, SL], BF16, tag="q")
        xnT_sb = persist.tile([128, CT, SL], BF16, tag="xnT")
        a_sb = persist.tile([128, H, SL], BF16, tag="a")

        # ---------------- AdaLN (sharded 768 cols/core) + e AllGather -------
        # All three sublayers' lora-down products y_L are computed first and
        # round-trip DRAM once; then the lora-up slices + temb -> e AllGather.
        with tc.tile_pool(name="adaln", bufs=3) as apool, \
             tc.tile_pool(name="adaln_ps", bufs=2, space="PSUM") as apsum:
            silu_sb = apool.tile([128, CT], BF16, tag="silu", bufs=1)
            nc.scalar.activation(out=silu_sb, in_=emb_sb, func=AF.Silu)
            temb_sb = apool.tile([1, 3, 256], F32, tag="tembl", bufs=1)
            nc.sync.dma_start(out=temb_sb, in_=bass.AP(
                tensor=temb_l.tensor, offset=0,
                ap=[[768, 1], [256, 3], [1, 256]]))
            e_loc = apool.tile([1, 9, 256], F32, tag="eloc", bufs=1)
            yall = apool.tile([1, 3, LORA], BF16, tag="yall", bufs=1)
            for L in range(3):
                aw1_sb = apool.tile([128, CT, LORA], BF16, tag="aw1")
                nc.sync.dma_start(
                    out=aw1_sb, in_=aw1[L].rearrange("(t p) m -> p t m",
                                                     p=128))
                y_ps = apsum.tile([1, LORA], F32, tag="yps")
                for ct in range(CT):
                    nc.tensor.matmul(y_ps, lhsT=silu_sb[:, ct:ct + 1],
                                     rhs=aw1_sb[:, ct, :],
                                     start=(ct == 0), stop=(ct == CT - 1))
                nc.scalar.copy(out=yall[:, L, :], in_=y_ps)
            # redistribute y to partitions via tiny PE transposes (no DRAM
            # round trip): y2[:, L, t2] = yall[0, L, 128*t2 : 128*(t2+1)]^T
            y2 = apool.tile([128, 3, 2], BF16, tag="y2", bufs=1)
            for L in range(3):
                for t2 in range(2):
                    y2p = apsum.tile([128, 1], F32, tag="y2p")
                    nc.tensor.matmul(
                        y2p, lhsT=yall[:, L, 128 * t2:128 * (t2 + 1)],
                        rhs=onecb_sb[0:1, :], start=True, stop=True)
                    nc.vector.tensor_copy(out=y2[:, L, t2:t2 + 1], in_=y2p)
            for L in range(3):
                aw2_sb = apool.tile([128, 2, 3, 256], BF16, tag="aw2")
                nc.sync.dma_start(
                    out=aw2_sb,
                    in_=aw2l[L].rearrange("(t p) (g n) -> p t g n",
                                          p=128, n=256))
                for g3 in range(3):
                    e_ps = apsum.tile([1, 256], F32, tag="eps2")
                    nc.tensor.matmul(e_ps, lhsT=y2[:, L, 0:1],
                                     rhs=aw2_sb[:, 0, g3, :],
                                     start=True, stop=False)
                    nc.tensor.matmul(e_ps, lhsT=y2[:, L, 1:2],
                                     rhs=aw2_sb[:, 1, g3, :],
                                     start=False, stop=True)
                    nc.vector.tensor_add(out=e_loc[:, 3 * L + g3, :],
                                         in0=e_ps,
                                         in1=temb_sb[:, g3, :])
            nc.sync.dma_start(out=bass.AP(
                tensor=e_in.tensor, offset=0,
                ap=[[2304, 1], [256, 9], [1, 256]]), in_=e_loc)
        nc.gpsimd.collective_compute(
            "AllGather", ALU.bypass, replica_groups=RG,
            ins=[e_in], outs=[e_out])

        nc.sync.dma_start(out=h_sb, in_=h_in.rearrange("(u p) c -> p u c",
                                                       p=128))
        nc.sync.dma_start(out=eye_sb, in_=eye)
        nc.sync.dma_start(out=onec_sb, in_=onec)
        nc.sync.dma_start(out=rotm_sb, in_=rotm)
        nc.sync.dma_start(out=cos2_sb, in_=bass.AP(
            tensor=cosT.tensor, offset=0, ap=[[SL, 128], [0, 2], [1, SL]]))
        nc.sync.dma_start(out=sin2_sb, in_=bass.AP(
            tensor=sinT.tensor, offset=0, ap=[[SL, 128], [0, 2], [1, SL]]))

        def _ebcast(L, t):
            # [128, 8, 256] broadcast view of e_out for (sublayer L, type t)
            return bass.AP(tensor=e_out.tensor, offset=(3 * L + t) * 256,
                           ap=[[0, 128], [9 * 256, 8], [1, 256]])

        # ---------------- modulate: LN + transpose + feature-major mod ------
        # LN and the transposes do NOT wait for the e AllGather; scale/shift
        # are applied per-partition (feature) on the transposed tiles.
        def modulate(L, mpool, mpsum):
            shT = mpool.tile([128, 2, 8], F32, tag="shT", bufs=1)
            opT = mpool.tile([128, 2, 8], F32, tag="opT", bufs=1)
            for par in range(2):
                nc.sync.dma_start(out=shT[:, par, :], in_=bass.AP(
                    tensor=e_out.tensor, offset=3 * L * 256 + par * 128,
                    ap=[[1, 128], [2304, 8]]))
                nc.sync.dma_start(out=opT[:, par, :], in_=bass.AP(
                    tensor=e_out.tensor, offset=(3 * L + 1) * 256 + par * 128,
                    ap=[[1, 128], [2304, 8]]))
            nc.vector.tensor_scalar_add(out=opT, in0=opT, scalar1=1.0)
            nc.sync.dma_start(out=gateB, in_=_ebcast(L, 2))
            for u in range(2):
                x = h_sb[:, u, :]
                stats = mpool.tile([128, 4, 6], F32, tag="stats")
                xv = x.rearrange("p (g n) -> p g n", n=512)
                for g in range(4):
                    nc.vector.bn_stats(out=stats[:, g, :], in_=xv[:, g, :])
                mv = mpool.tile([128, 2], F32, tag="mv")
                nc.vector.bn_aggr(out=mv, in_=stats)
                sd = mpool.tile([128, 1], F32, tag="sd")
                nc.scalar.activation(out=sd, in_=mv[:, 1:2], func=AF.Sqrt,
                                     bias=eps_sb)
                rstd = mpool.tile([128, 1], F32, tag="rstd")
                nc.vector.reciprocal(out=rstd, in_=sd)
                t1 = mpool.tile([128, C], F32R, tag="mod_t1",
                                name=f"t1_{u}")
                nc.vector.tensor_scalar(out=t1, in0=x, scalar1=mv[:, 0:1],
                                        scalar2=rstd, op0=ALU.subtract,
                                        op1=ALU.mult)
                for ct in range(CT):
                    tp = mpsum.tile([128, 128], F32R, tag="tpsum")
                    nc.tensor.transpose(
                        tp, t1[:, 128 * ct:128 * (ct + 1)], eye_sb)
                    nc.vector.tensor_scalar(
                        out=xnT_sb[:, ct, 128 * u:128 * (u + 1)], in0=tp,
                        scalar1=opT[:, ct % 2, ct // 2:ct // 2 + 1],
                        scalar2=shT[:, ct % 2, ct // 2:ct // 2 + 1],
                        op0=ALU.mult, op1=ALU.add)

        # batched rms(+rope) epilogue: src sbuf f32r [128, 2, 256] (2 heads)
        def rms_ep(src, dst, rope, pool, psum):
            sq = pool.tile([128, 2 * SL], F32R, tag="ep_sq")
            nc.scalar.activation(out=sq, in_=src, func=AF.Square)
            ss = psum.tile([1, 2 * SL], F32, tag="ep_ss")
            nc.tensor.matmul(ss, lhsT=onec_sb, rhs=sq, start=True, stop=True)
            sd = pool.tile([1, 2 * SL], F32, tag="ep_sd")
            nc.scalar.activation(out=sd, in_=ss, func=AF.Sqrt, scale=1.0 / D,
                                 bias=eps_sb[0:1, :])
            rc = pool.tile([1, 2 * SL], F32, tag="ep_rc")
            nc.vector.reciprocal(out=rc, in_=sd)
            rb = pool.tile([128, 2 * SL], F32, tag="ep_rb")
            nc.gpsimd.partition_broadcast(out_ap=rb, in_ap=rc)
            if rope:
                rot = psum.tile([128, 2 * SL], F32, tag="ep_rot")
                nc.tensor.matmul(rot, lhsT=rotm_sb, rhs=src,
                                 start=True, stop=True)
                t1 = pool.tile([128, 2 * SL], F32, tag="ep_t1")
                nc.vector.tensor_mul(out=t1, in0=src, in1=cos2_sb)
                t2 = pool.tile([128, 2 * SL], F32, tag="ep_t2")
                nc.vector.tensor_mul(out=t2, in0=rot, in1=sin2_sb)
                nc.vector.tensor_add(out=t1, in0=t1, in1=t2)
                nc.vector.tensor_mul(out=dst, in0=t1, in1=rb)
            else:
                nc.vector.tensor_mul(out=dst, in0=src, in1=rb)

        # wo + gated residual (shared by attn1/attn2); first chunks may
        # be prefetched into `pre` while the preceding phase runs
        def wo_prefetch(wX, pool, n=2):
            wv_ = wX.rearrange("(t p) (cc m) -> p cc t m", p=128, m=512)
            pre = []
            for cc in range(n):
                wo_sb = pool.tile([128, CT, 512], BF16, tag="wos")
                nc.sync.dma_start(out=wo_sb, in_=wv_[:, cc])
                pre.append(wo_sb)
            return wv_, pre

        def wo_residual(wv_, pool, pre):
            with tc.tile_pool(name="wops", bufs=2, space="PSUM") as psum:
                for cc in range(4):
                    if cc < len(pre):
                        wo_sb = pre[cc]
                    else:
                        wo_sb = pool.tile([128, CT, 512], BF16, tag="wos")
                        nc.sync.dma_start(out=wo_sb, in_=wv_[:, cc])
                    for u in range(2):
                        ops = psum.tile([128, 512], F32, tag="wops")
                        for hh in range(H):
                            nc.tensor.matmul(
                                ops,
                                lhsT=a_sb[:, hh, 128 * u:128 * (u + 1)],
                                rhs=wo_sb[:, hh, :],
                                start=(hh == 0), stop=(hh == H - 1))
                        gt = pool.tile([128, 512], F32, tag="wogt")
                        nc.vector.tensor_mul(
                            out=gt, in0=ops,
                            in1=gateB[:, 512 * cc:512 * (cc + 1)])
                        nc.vector.tensor_add(
                            out=h_sb[:, u, 512 * cc:512 * (cc + 1)],
                            in0=h_sb[:, u, 512 * cc:512 * (cc + 1)], in1=gt)

        # ======================= sublayer 0: self attention ================
        with tc.tile_pool(name="mod0", bufs=2) as mpool, \
             tc.tile_pool(name="mod0ps", bufs=2, space="PSUM") as mpsum:
            modulate(0, mpool, mpsum)

        # K/V in 4 head-groups of 4, written to k1_in/v1_in, then AllGathers
        wq1v = wq1.rearrange("(t p) (g m) -> p g t m", p=128, m=512)
        wk1v = wk1.rearrange("(t p) (g m) -> p g t m", p=128, m=512)
        wv1v = wv1.rearrange("(t p) (g m) -> p g t m", p=128, m=512)
        with tc.tile_pool(name="kv1w", bufs=2) as wpool, \
             tc.tile_pool(name="kv1e", bufs=1) as epool, \
             tc.tile_pool(name="kv1ps", bufs=1, space="PSUM") as qpsum, \
             tc.tile_pool(name="kv1ps2", bufs=1, space="PSUM") as qpsum2:
            # K pass first: the long rms+rope epilogue chains then overlap
            # the V matmul pass, so the AllGather is gated only by V's short
            # eviction chain.
            for g in range(4):
                wk_sb = wpool.tile([128, CT, 512], BF16, tag="wk")
                nc.sync.dma_start(out=wk_sb, in_=wk1v[:, g])
                pk = qpsum.tile([128, 4, SL], F32, tag="pk", bufs=2)
                for ct in range(CT):
                    st, sp = (ct == 0), (ct == CT - 1)
                    for j in range(4):
                        nc.tensor.matmul(
                            pk[:, j, :],
                            lhsT=wk_sb[:, ct, 128 * j:128 * (j + 1)],
                            rhs=xnT_sb[:, ct, :], start=st, stop=sp)
                kf = epool.tile([128, 4, SL], F32R, tag="kf")
                nc.vector.tensor_copy(out=kf, in_=pk)
                for jj in range(2):
                    ke = epool.tile([128, 2, SL], BF16, tag="ke")
                    rms_ep(kf[:, 2 * jj:2 * jj + 2, :], ke, True,
                           epool, qpsum2)
                    nc.sync.dma_start(
                        out=bass.AP(
                            tensor=kv1_in.tensor,
                            offset=(4 * g + 2 * jj) * D * SL,
                            ap=[[SL, 128], [D * SL, 2], [1, SL]]),
                        in_=ke)
            for g in range(4):
                wv_sb = wpool.tile([128, CT, 512], BF16, tag="wv")
                nc.sync.dma_start(out=wv_sb, in_=wv1v[:, g])
                pv = qpsum.tile([128, 2, 512], F32, tag="pv")
                for ct in range(CT):
                    st, sp = (ct == 0), (ct == CT - 1)
                    for u in range(2):
                        nc.tensor.matmul(
                            pv[:, u, :],
                            lhsT=xnT_sb[:, ct, 128 * u:128 * (u + 1)],
                            rhs=wv_sb[:, ct, :], start=st, stop=sp)
                ve = epool.tile([128, 2, 512], BF16, tag="ve")
                nc.scalar.copy(out=ve, in_=pv)
                nc.sync.dma_start(
                    out=bass.AP(tensor=kv1_in.tensor,
                                offset=KV1K + 512 * g,
                                ap=[[C, 128], [128 * C, 2], [1, 512]]),
                    in_=ve)
        nc.gpsimd.collective_compute(
            "AllGather", ALU.bypass, replica_groups=RG,
            ins=[kv1_in], outs=[kv1_out])

        # ---- during the AllGather window: Q (rms+rope) and enc K2/V2 ------
        with tc.tile_pool(name="q1w", bufs=2) as wpool, \
             tc.tile_pool(name="q1e", bufs=1) as epool, \
             tc.tile_pool(name="q1ps", bufs=1, space="PSUM") as qpsum, \
             tc.tile_pool(name="q1ps2", bufs=1, space="PSUM") as qpsum2:
            for g in range(4):
                wq_sb = wpool.tile([128, CT, 512], BF16, tag="wq")
                nc.scalar.dma_start(out=wq_sb, in_=wq1v[:, g])
                pq = qpsum.tile([128, 4, SL], F32, tag="pq")
                for ct in range(CT):
                    st, sp = (ct == 0), (ct == CT - 1)
                    for j in range(4):
                        nc.tensor.matmul(
                            pq[:, j, :],
                            lhsT=wq_sb[:, ct, 128 * j:128 * (j + 1)],
                            rhs=xnT_sb[:, ct, :], start=st, stop=sp)
                qf = epool.tile([128, 4, SL], F32R, tag="qf")
                nc.vector.tensor_copy(out=qf, in_=pq)
                for jj in range(2):
                    h0 = 4 * g + 2 * jj
                    rms_ep(qf[:, 2 * jj:2 * jj + 2, :],
                           q_sb[:, h0:h0 + 2, :], True, epool, qpsum2)

        # replicated cross-attn K2^T/V2 from the encoder (all 16 heads);
        # results are spilled to DRAM and reloaded at sdpa2 to keep SBUF free
        wk2v = wk2.rearrange("(t p) (g m) -> p g t m", p=128, m=256)
        wv2v = wv2.rearrange("(t p) (g m) -> p g t m", p=128, m=256)
        with tc.tile_pool(name="enckv", bufs=2) as kpool, \
             tc.tile_pool(name="enckvps", bufs=1, space="PSUM") as kpsum, \
             tc.tile_pool(name="enckvps2", bufs=1, space="PSUM") as kpsum2:
            k2_sb = kpool.tile([128, H, SC], BF16, tag="k2", bufs=1)
            v2_sb = kpool.tile([128, 4, H, D], BF16, tag="v2", bufs=1)
            enc_sb = kpool.tile([128, 8, SC], BF16, tag="enc", bufs=1)
            nc.scalar.dma_start(out=enc_sb,
                              in_=encT.rearrange("(t p) s -> p t s", p=128))
            for g in range(8):  # pairs of heads
                wk2_sb = kpool.tile([128, 8, 256], BF16, tag="wk2")
                nc.scalar.dma_start(out=wk2_sb, in_=wk2v[:, g])
                wv2_sb = kpool.tile([128, 8, 256], BF16, tag="wv2")
                nc.scalar.dma_start(out=wv2_sb, in_=wv2v[:, g])
                pk2 = kpsum.tile([128, 2, SC], F32, tag="pk2")
                pv2 = kpsum.tile([128, 4, 256], F32, tag="pv2")
                for t in range(8):
                    st, sp = (t == 0), (t == 7)
                    for j in range(2):
                        nc.tensor.matmul(
                            pk2[:, j, :],
                            lhsT=wk2_sb[:, t, 128 * j:128 * (j + 1)],
                            rhs=enc_sb[:, t, :], start=st, stop=sp)
                    for tt in range(4):
                        nc.tensor.matmul(
                            pv2[:, tt, :],
                            lhsT=enc_sb[:, t, 128 * tt:128 * (tt + 1)],
                            rhs=wv2_sb[:, t, :], start=st, stop=sp)
                kf2 = kpool.tile([128, 2, SC], F32R, tag="kf2")
                nc.vector.tensor_copy(out=kf2, in_=pk2)
                # batched rms over head dim (partition), no rope
                sq = kpool.tile([128, 2, SC], F32R, tag="k2sq")
                nc.scalar.activation(out=sq, in_=kf2, func=AF.Square)
                ss = kpsum2.tile([1, 2, SC], F32, tag="k2ss")
                for j in range(2):
                    nc.tensor.matmul(ss[:, j, :], lhsT=onec_sb,
                                     rhs=sq[:, j, :], start=True, stop=True)
                sd = kpool.tile([1, 2 * SC], F32, tag="k2sd")
                nc.scalar.activation(out=sd, in_=ss, func=AF.Sqrt,
                                     scale=1.0 / D, bias=eps_sb[0:1, :])
                rc = kpool.tile([1, 2 * SC], F32, tag="k2rc")
                nc.vector.reciprocal(out=rc, in_=sd)
                rb = kpool.tile([128, 2 * SC], F32, tag="k2rb")
                nc.gpsimd.partition_broadcast(out_ap=rb, in_ap=rc)
                nc.vector.tensor_mul(out=k2_sb[:, 2 * g:2 * g + 2, :],
                                     in0=kf2, in1=rb)
                nc.vector.tensor_copy(out=v2_sb[:, :, 2 * g:2 * g + 2, :],
                                      in_=pv2)
            nc.sync.dma_start(out=bass.AP(
                tensor=kv2d.tensor, offset=0,
                ap=[[H * SC, 128], [1, H * SC]]), in_=k2_sb)
            nc.sync.dma_start(out=bass.AP(
                tensor=kv2d.tensor, offset=128 * H * SC,
                ap=[[H * SC, 128], [1, H * SC]]), in_=v2_sb)

        # sdpa over 2 halves of 8 heads; 4-head interleaved softmax rounds.
        # PE emission is software-pipelined: scores(kt) are issued before
        # den/av(kt-1) so PE never stalls on the Exp of the current round.
        def sdpa1(spool, spsum, spsum2):
            for quarter in range(4):
                hb0 = 4 * quarter
                k_sb = spool.tile([128, 4, 8, SL], BF16, tag="ksb", bufs=2)
                v_sb = spool.tile([128, 16, 512], BF16, tag="vsb", bufs=2)
                for r in range(NCORES):
                    nc.sync.dma_start(out=k_sb[:, :, r, :], in_=bass.AP(
                        tensor=kv1_out.tensor,
                        offset=hb0 * D * SL + r * KV1SZ,
                        ap=[[SL, 128], [D * SL, 4], [1, SL]]))
                    nc.sync.dma_start(out=v_sb[:, 2 * r:2 * r + 2, :],
                                      in_=bass.AP(
                        tensor=kv1_out.tensor,
                        offset=KV1K + hb0 * D + r * KV1SZ,
                        ap=[[C, 128], [128 * C, 2], [1, 512]]))
                if True:
                    hb = 0
                    av = spsum.tile([128, 4, SL], F32, tag="av")
                    den = spsum.tile([1, 2, 2 * SL], F32, tag="den")

                    def sc_round(kt):
                        sps = spsum2.tile([128, 4, SL], F32, tag="sps",
                                          name=f"sps{kt}")
                        for j in range(4):
                            nc.tensor.matmul(
                                sps[:, j, :],
                                lhsT=k_sb[:, j, kt // 2,
                                          128 * (kt % 2):128 * (kt % 2) + 128],
                                rhs=q_sb[:, hb0 + j, :],
                                start=True, stop=True)
                        pt = spool.tile([128, 4, SL], BF16, tag="pt",
                                        name=f"pt{kt}")
                        nc.scalar.activation(out=pt, in_=sps, func=AF.Exp,
                                             scale=ISQD)
                        return pt

                    def av_round(kt, pt):
                        st, sp = (kt == 0), (kt == 15)
                        for dh in range(2):
                            nc.tensor.matmul(
                                den[:, dh, :], lhsT=onecb_sb,
                                rhs=pt[:, 2 * dh:2 * dh + 2, :],
                                start=st, stop=sp)
                        for j in range(4):
                            nc.tensor.matmul(
                                av[:, j, :],
                                lhsT=v_sb[:, kt, 128 * j:128 * (j + 1)],
                                rhs=pt[:, j, :], start=st, stop=sp)

                    prev = sc_round(0)
                    for kt in range(1, 16):
                        cur = sc_round(kt)
                        av_round(kt - 1, prev)
                        prev = cur
                    av_round(15, prev)
                    rc = spool.tile([1, 4 * SL], F32, tag="sd_rc")
                    nc.vector.reciprocal(out=rc, in_=den)
                    rb = spool.tile([128, 4 * SL], F32, tag="sd_rb")
                    nc.gpsimd.partition_broadcast(out_ap=rb, in_ap=rc)
                    nc.vector.tensor_mul(
                        out=a_sb[:, hb0:hb0 + 4, :],
                        in0=av, in1=rb)

        wo1_ctx = tc.tile_pool(name="wo1", bufs=2)
        wo1_pool = wo1_ctx.__enter__()
        wo1v, wo1_pre = wo_prefetch(wo1, wo1_pool)

        with tc.tile_pool(name="sdpa1", bufs=3) as spool, \
             tc.tile_pool(name="sdpa1ps", bufs=1, space="PSUM") as spsum, \
             tc.tile_pool(name="sdpa1ps2", bufs=2, space="PSUM") as spsum2:
            sdpa1(spool, spsum, spsum2)

        wo_residual(wo1v, wo1_pool, wo1_pre)
        wo1_ctx.__exit__(None, None, None)

        # ======================= sublayer 1: cross attention ===============
        with tc.tile_pool(name="mod1", bufs=2) as mpool, \
             tc.tile_pool(name="mod1ps", bufs=2, space="PSUM") as mpsum:
            modulate(1, mpool, mpsum)

        wq2v = wq2.rearrange("(t p) (g m) -> p g t m", p=128, m=512)
        with tc.tile_pool(name="q2w", bufs=2) as wpool, \
             tc.tile_pool(name="q2e", bufs=1) as epool, \
             tc.tile_pool(name="q2ps", bufs=1, space="PSUM") as qpsum, \
             tc.tile_pool(name="q2ps2", bufs=1, space="PSUM") as qpsum2:
            for g in range(4):
                wq_sb = wpool.tile([128, CT, 512], BF16, tag="wq2")
                nc.scalar.dma_start(out=wq_sb, in_=wq2v[:, g])
                pq = qpsum.tile([128, 4, SL], F32, tag="pq2")
                for ct in range(CT):
                    st, sp = (ct == 0), (ct == CT - 1)
                    for j in range(4):
                        nc.tensor.matmul(
                            pq[:, j, :],
                            lhsT=wq_sb[:, ct, 128 * j:128 * (j + 1)],
                            rhs=xnT_sb[:, ct, :], start=st, stop=sp)
                qf = epool.tile([128, 4, SL], F32R, tag="qf2")
                nc.vector.tensor_copy(out=qf, in_=pq)
                for jj in range(2):
                    h0 = 4 * g + 2 * jj
                    rms_ep(qf[:, 2 * jj:2 * jj + 2, :],
                           q_sb[:, h0:h0 + 2, :], False, epool, qpsum2)

        def sdpa2(spool, spsum, spsum2):
            k2_sb = spool.tile([128, H, SC], BF16, tag="k2r", bufs=1)
            nc.sync.dma_start(out=k2_sb, in_=bass.AP(
                tensor=kv2d.tensor, offset=0,
                ap=[[H * SC, 128], [1, H * SC]]))
            v2_sb = spool.tile([128, 4, H, D], BF16, tag="v2r", bufs=1)
            nc.sync.dma_start(out=v2_sb, in_=bass.AP(
                tensor=kv2d.tensor, offset=128 * H * SC,
                ap=[[H * SC, 128], [1, H * SC]]))
            for sg in range(4):
                hb = 4 * sg
                av = spsum.tile([128, 4, SL], F32, tag="av2")
                den = spsum.tile([1, 2, 2 * SL], F32, tag="den2")

                def sc_round(kt):
                    sps = spsum2.tile([128, 4, SL], F32, tag="sps2",
                                      name=f"sps2_{kt}")
                    for j in range(4):
                        nc.tensor.matmul(
                            sps[:, j, :],
                            lhsT=k2_sb[:, hb + j, 128 * kt:128 * (kt + 1)],
                            rhs=q_sb[:, hb + j, :], start=True, stop=True)
                    pt = spool.tile([128, 4, SL], BF16, tag="pt2",
                                    name=f"pt2_{kt}")
                    nc.scalar.activation(out=pt, in_=sps, func=AF.Exp,
                                         scale=ISQD)
                    return pt

                def av_round(kt, pt):
                    st, sp = (kt == 0), (kt == 3)
                    for dh in range(2):
                        nc.tensor.matmul(
                            den[:, dh, :], lhsT=onecb_sb,
                            rhs=pt[:, 2 * dh:2 * dh + 2, :],
                            start=st, stop=sp)
                    for j in range(4):
                        nc.tensor.matmul(
                            av[:, j, :],
                            lhsT=v2_sb[:, kt, hb + j, :],
                            rhs=pt[:, j, :], start=st, stop=sp)

                prev = sc_round(0)
                for kt in range(1, 4):
                    cur = sc_round(kt)
                    av_round(kt - 1, prev)
                    prev = cur
                av_round(3, prev)
                rc = spool.tile([1, 4 * SL], F32, tag="sd2_rc")
                nc.vector.reciprocal(out=rc, in_=den)
                rb = spool.tile([128, 4 * SL], F32, tag="sd2_rb")
                nc.gpsimd.partition_broadcast(out_ap=rb, in_ap=rc)
                nc.vector.tensor_mul(out=a_sb[:, hb:hb + 4, :],
                                     in0=av, in1=rb)

        w1v = w1f.rearrange("(t p) (fg m) -> p fg t m", p=128, m=512)
        ff1_ctx = tc.tile_pool(name="ff1", bufs=2)
        ff1_pool = ff1_ctx.__enter__()
        wo2_ctx = tc.tile_pool(name="wo2", bufs=2)
        wo2_pool = wo2_ctx.__enter__()
        wo2v, wo2_pre = wo_prefetch(wo2, wo2_pool)
        w1_pre = []
        for fg in range(2):
            w1_sb = ff1_pool.tile([128, CT, 512], BF16, tag="w1")
            nc.sync.dma_start(out=w1_sb, in_=w1v[:, fg])
            w1_pre.append(w1_sb)

        with tc.tile_pool(name="sdpa2", bufs=3) as spool, \
             tc.tile_pool(name="sdpa2ps", bufs=1, space="PSUM") as spsum, \
             tc.tile_pool(name="sdpa2ps2", bufs=2, space="PSUM") as spsum2:
            sdpa2(spool, spsum, spsum2)

        wo_residual(wo2v, wo2_pool, wo2_pre)
        wo2_ctx.__exit__(None, None, None)

        # ======================= sublayer 2: feed forward ==================
        with tc.tile_pool(name="mod2", bufs=2) as mpool, \
             tc.tile_pool(name="mod2ps", bufs=2, space="PSUM") as mpsum:
            modulate(2, mpool, mpsum)

        w2v = w2f.rearrange("(f p) (cc m) -> p cc f m", p=128, m=256)
        with tc.tile_pool(name="ffg", bufs=1) as gpool:
            g_sb = gpool.tile([128, FT, SL], BF16, tag="g")
            with tc.tile_pool(name="ff1ps", bufs=4, space="PSUM") as fpsum:
                for fg in range(16):
                    if fg < len(w1_pre):
                        w1_sb = w1_pre[fg]
                    else:
                        w1_sb = ff1_pool.tile([128, CT, 512], BF16, tag="w1")
                        nc.sync.dma_start(out=w1_sb, in_=w1v[:, fg])
                    for ft in range(4):
                        gps = fpsum.tile([128, SL], F32, tag="gps")
                        for ct in range(CT):
                            nc.tensor.matmul(
                                gps,
                                lhsT=w1_sb[:, ct, 128 * ft:128 * (ft + 1)],
                                rhs=xnT_sb[:, ct, :],
                                start=(ct == 0), stop=(ct == CT - 1))
                        nc.scalar.activation(
                            out=g_sb[:, 4 * fg + ft, :], in_=gps,
                            func=AF.Gelu)
            with tc.tile_pool(name="ff2", bufs=2) as wpool, \
                 tc.tile_pool(name="ff2ps", bufs=2, space="PSUM") as fpsum:
                for cc in range(8):
                    w2_sb = wpool.tile([128, FT, 256], BF16, tag="w2")
                    nc.sync.dma_start(out=w2_sb, in_=w2v[:, cc])
                    for u in range(2):
                        yps = fpsum.tile([128, 256], F32, tag="yps")
                        for f in range(FT):
                            nc.tensor.matmul(
                                yps,
                                lhsT=g_sb[:, f, 128 * u:128 * (u + 1)],
                                rhs=w2_sb[:, f, :],
                                start=(f == 0), stop=(f == FT - 1))
                        gt = wpool.tile([128, 256], F32, tag="ffgt")
                        nc.vector.tensor_mul(
                            out=gt, in0=yps,
                            in1=gateB[:, 256 * cc:256 * (cc + 1)])
                        nc.vector.tensor_add(
                            out=h_sb[:, u, 256 * cc:256 * (cc + 1)],
                            in0=h_sb[:, u, 256 * cc:256 * (cc + 1)], in1=gt)
                        nc.sync.dma_start(
                            out=bass.AP(tensor=h_out.tensor,
                                        offset=u * 128 * C + 256 * cc,
                                        ap=[[C, 128], [1, 256]]),
                            in_=h_sb[:, u, 256 * cc:256 * (cc + 1)])

        ff1_ctx.__exit__(None, None, None)

        ctx_pool.__exit__(None, None, None)

    nc.compile()
    return nc


_NC_CACHE = None


def _get_nc():
    global _NC_CACHE
    if _NC_CACHE is None:
        nc = bacc.Bacc("TRN2", target_bir_lowering=False, debug=False,
                       num_devices=NCORES)
        _NC_CACHE = _build(nc)
    return _NC_CACHE


def _bf(a):
    return np.ascontiguousarray(np.asarray(a, np.float32)).astype(
        ml_dtypes.bfloat16)


def kernel(**inputs) -> np.ndarray:
    h = np.asarray(inputs["hidden_states"], np.float32)[0]      # [S, C]
    enc = np.asarray(inputs["encoder_hidden_states"], np.float32)[0]
    emb = np.asarray(inputs["embedded_timestep"], np.float32)[0]  # [C]
    temb = np.asarray(inputs["temb"], np.float32)                # [1, 3C]
    cosT = np.ascontiguousarray(np.asarray(inputs["rope_cos"],
                                           np.float32).T)        # [D, S]
    sinT = np.ascontiguousarray(np.asarray(inputs["rope_sin"],
                                           np.float32).T)

    for k in ("attn1_qn", "attn1_kn", "attn2_qn", "attn2_kn"):
        assert np.allclose(np.asarray(inputs[k]), 1.0), f"{k} != ones"

    rot = np.zeros((D, D), np.float32)  # rot_out = rot_m.T @ q
    for d in range(D // 2):
        rot[64 + d, d] = -1.0
        rot[d, 64 + d] = 1.0

    aw2 = [np.asarray(inputs[f"a{i}_w2"], np.float32) for i in (1, 2, 3)]
    common = {
        "wq1": _bf(inputs["attn1_wq"]), "wk1": _bf(inputs["attn1_wk"]),
        "wv1": _bf(inputs["attn1_wv"]), "wo1": _bf(inputs["attn1_wo"]),
        "wq2": _bf(inputs["attn2_wq"]), "wo2": _bf(inputs["attn2_wo"]),
        "wk2": _bf(inputs["attn2_wk"]), "wv2": _bf(inputs["attn2_wv"]),
        "w1f": _bf(inputs["ff_w1"]), "w2f": _bf(inputs["ff_w2"]),
        "encT": _bf(enc.T),
        "aw1": _bf(np.stack([inputs["a1_w1"], inputs["a2_w1"],
                             inputs["a3_w1"]])),
        "emb_t": np.ascontiguousarray(emb.reshape(CT, 128).T),
        "rotm": round_fp32r(rot),
        "eye": round_fp32r(np.eye(128, dtype=np.float32)),
        "onec": np.ones((128, 1), np.float32),
        "onecb": np.ones((128, 1), np.float32).astype(ml_dtypes.bfloat16),
    }
    in_maps = []
    for r in range(NCORES):
        tok = slice(SL * r, SL * (r + 1))
        hd = slice(256 * r, 256 * (r + 1))
        m = dict(common)
        m["h_s"] = np.ascontiguousarray(h[tok, :])
        m["cosT"] = np.ascontiguousarray(cosT[:, tok])
        m["sinT"] = np.ascontiguousarray(sinT[:, tok])
        m["aw2l"] = _bf(np.stack([
            np.concatenate([a[:, 2048 * t + 256 * r:2048 * t + 256 * (r + 1)]
                            for t in range(3)], axis=1) for a in aw2]))
        m["temb_l"] = np.ascontiguousarray(np.stack(
            [temb[0, 2048 * t + 256 * r:2048 * t + 256 * (r + 1)]
             for t in range(3)]))
        in_maps.append(m)

    nc = _get_nc()
    res = run_bass_kernel_spmd(nc, in_maps, core_ids=list(range(NCORES)))
    out = np.concatenate([res.results[i]["h_out"] for i in range(NCORES)],
                         axis=0)
    return out.reshape(1, S, C).astype(np.float32)


if __name__ == "__main__":
    _get_nc()
    print("build + compile OK")
